# revision 1
# baseline (speedup 1.0000x reference)
"""GraphUNet (N=4096, E=65536, C=256, depth 3, ratio 0.5) on 8 trn2 NeuronCores.

Row-sharded SPMD pipeline, 6 launches; the host only does top-k, gathers,
degree/scaling-vector prep and small C x C weight folds between launches:

  K1   init GCN:  x0_rows = N0[rows] @ (x @ Wi), N0 = D(A0+2I)D host-built
  K2   level 1:   M^T col-block per core = (Bh[:,perm])^T-chain in fp8 with
                  DoubleRow (adjacency entries are small ints -> exact),
                  scaled by dis vectors on device -> N1^T block; diag error
                  folded into the host `h` correction; then the down-GCN
                  (float32r) + relu. N^T blocks ship back as bf16 (exact).
  K3   level 2:   same at n=1024 (fp8).
  K4a  level 3:   same at n=512 (bf16, entries <= 69).
  K4b  up GCNs:   xU1 = relu(N2 @ (xa1 @ Wu0)) with Wu0 host-folded via
                  associativity (removes all transposes); gcn1up sharded,
                  unpool-scatter folded into host-gathered lhsT N1[:,perm1].
  K4c  final GCN: out_rows = P0^T @ (x0@Wf) + Q0^T @ (xU2@Wf); dis and the
                  scatter are folded into host-prepped P0/Q0, Wf host-folded.

Precision: down-path (top-k-relevant) in float32r (~1e-4, safe: measured
output sensitivity to boundary flips is tiny); post-top-k path in bf16.
Integer adjacency matmuls in fp8/bf16 are exact.
"""

import numpy as np
import ml_dtypes

from contextlib import ExitStack

import concourse.bass as bass
import concourse.mybir as mybir
import concourse.tile as tile
from concourse import bacc
from concourse.bass_utils import run_bass_kernel_spmd

NCORES = 8
C = 256
F32 = mybir.dt.float32
F32R = mybir.dt.float32r
BF16 = mybir.dt.bfloat16
FP8 = mybir.dt.float8e4

NP_OF = {F32: np.float32, F32R: np.float32,
         BF16: ml_dtypes.bfloat16, FP8: ml_dtypes.float8_e4m3fn}

_TRACE = {"on": False, "results": [], "ncs": []}
_CHUNK_BYTES = 2 << 20


def _r3(ap, p=128):
    """[K, F] dram view -> [p, K//p, F] (partition, ktile, free)."""
    return ap.rearrange("(o p) f -> p o f", p=p)


def _load(nc, pool, dram, name):
    """Load [K, F] dram into a [128, K//128, F] sbuf tile, chunking large
    transfers so downstream matmuls can start on early k-tiles."""
    K, F = dram.shape
    if K % 128 == 0:
        KT = K // 128
        t = pool.tile([128, KT, F], dram.dtype, tag=name)
        r = _r3(dram.ap())
        nbytes = K * F * mybir.dt.size(dram.dtype)
        nchunks = min(KT, max(1, nbytes // _CHUNK_BYTES))
        step = (KT + nchunks - 1) // nchunks
        for k0 in range(0, KT, step):
            k1 = min(KT, k0 + step)
            nc.sync.dma_start(t[:, k0:k1, :], r[:, k0:k1, :])
    else:
        assert K < 128, (name, K)
        t = pool.tile([128, 1, F], dram.dtype, tag=name)
        nc.sync.dma_start(t[:K, 0, :], dram.ap())
    return t


def _mm_block(nc, psum_pool, chains, M, NF, consumer, tagp="ps"):
    """out[M, NF] = sum over chains of lhsT.T @ rhs, yielding per-128-row
    psum tiles to consumer(mo, ps). chains: [(lhsT3d, rhs3d, KT)]."""
    total = sum(kt for _, _, kt in chains)
    n_mo = (M + 127) // 128
    for mo in range(n_mo):
        msz = min(128, M - mo * 128)
        ps = psum_pool.tile([128, NF], F32, tag=tagp)
        cnt = 0
        for lhsT, rhs, KT in chains:
            # fp8 DoubleRow: pack 2 k-tiles per matmul (2x PE throughput)
            use_dr = (lhsT.dtype == FP8 and rhs.dtype == FP8
                      and KT % 2 == 0 and msz == 128)
            if use_dr:
                for kp in range(KT // 2):
                    cnt += 2
                    nc.tensor.matmul(
                        ps[:msz, :],
                        lhsT[:, 2 * kp:2 * kp + 2, mo * 128:mo * 128 + msz],
                        rhs[:, 2 * kp:2 * kp + 2, :],
                        start=(cnt == 2), stop=(cnt == total),
                        perf_mode=mybir.MatmulPerfMode.DoubleRow)
            else:
                for kt in range(KT):
                    cnt += 1
                    nc.tensor.matmul(
                        ps[:msz, :], lhsT[:, kt, mo * 128:mo * 128 + msz],
                        rhs[:, kt, :], start=(cnt == 1), stop=(cnt == total))
        consumer(mo, ps[:msz, :])



def _mm_block_ko(nc, psum_pool, chains, M, NF, consumer, tagp="pko"):
    """kt-outer variant of _mm_block: all row-block psums live at once, so
    each arriving k-chunk's matmuls fire immediately. Use when M//128 <= 4."""
    total = sum(kt for _, _, kt in chains)
    n_mo = (M + 127) // 128
    pss = [psum_pool.tile([128, NF], F32, tag=f"{tagp}{i}",
                          name=f"{tagp}{i}")
           for i in range(n_mo)]
    cnt = 0
    for lhsT, rhs, KT in chains:
        for kt in range(KT):
            cnt += 1
            for mo in range(n_mo):
                msz = min(128, M - mo * 128)
                nc.tensor.matmul(
                    pss[mo][:msz, :], lhsT[:, kt, mo * 128:mo * 128 + msz],
                    rhs[:, kt, :], start=(cnt == 1), stop=(cnt == total))
    for mo in range(n_mo):
        msz = min(128, M - mo * 128)
        consumer(mo, pss[mo][:msz, :])


def _transpose_block(nc, sb_pool, psum_pool, ident, v_sb, MT, name):
    """v_sb [128, MT, C] f32r -> vT [128, C//128, MT*128] f32r."""
    vT = sb_pool.tile([128, C // 128, MT * 128], v_sb.dtype, tag=name)
    for mo in range(MT):
        for cc in range(C // 128):
            pst = psum_pool.tile([128, 128], v_sb.dtype, tag="pst")
            nc.tensor.transpose(pst[:], v_sb[:, mo, cc * 128:(cc + 1) * 128],
                                ident[:])
            nc.any.tensor_copy(vT[:, cc, mo * 128:(mo + 1) * 128], pst[:])
    return vT


def _new_nc():
    return bacc.Bacc("TRN2", target_bir_lowering=False, debug=False,
                     num_devices=NCORES)


def _finish(nc):
    nc.compile()
    _TRACE["ncs"].append(nc)
    return nc


def _run(nc, in_maps):
    res = run_bass_kernel_spmd(nc, in_maps, list(range(NCORES)),
                               trace=_TRACE["on"])
    if _TRACE["on"]:
        _TRACE["results"].append(res)
    return res.results


# ------------------------------------------------------------------ K1
def build_k1(n, rpc):
    nc = _new_nc()
    xT = nc.dram_tensor("xT", [C, n], F32R, kind="ExternalInput")
    Wi = nc.dram_tensor("Wi", [C, C], F32R, kind="ExternalInput")
    NT0 = nc.dram_tensor("NT0", [n, rpc], F32R, kind="ExternalInput")
    xo = nc.dram_tensor("xo", [rpc, C], F32, kind="ExternalOutput")
    with tile.TileContext(nc) as tc:
        ctx = ExitStack()
        sb = ctx.enter_context(tc.tile_pool(name="sb", bufs=1))
        ps = ctx.enter_context(tc.tile_pool(name="ps", bufs=8, space="PSUM"))
        xT_sb = _load(nc, sb, xT, "xT")
        Wi_sb = _load(nc, sb, Wi, "Wi")
        NT0_sb = _load(nc, sb, NT0, "NT0")
        y0 = sb.tile([128, n // 128, C], F32R, tag="y0")
        _mm_block(nc, ps, [(xT_sb, Wi_sb, C // 128)], n, C,
                  lambda mo, p: nc.any.tensor_copy(y0[:, mo, :], p))
        xo_sb = sb.tile([128, rpc // 128, C], F32, tag="xo")
        _mm_block(nc, ps, [(NT0_sb, y0, n // 128)], rpc, C,
                  lambda mo, p: nc.any.tensor_copy(xo_sb[:, mo, :], p))
        nc.sync.dma_start(_r3(xo.ap()), xo_sb[:])
        ctx.close()
    return _finish(nc)


# ------------------------------------------------------- K2 / K3 / K4a
def build_level(n_prev, n, rpc, adt, want_nt_out, want_relu=True):
    nc = _new_nc()
    R = nc.dram_tensor("R", [n_prev, n], adt, kind="ExternalInput")
    LT = nc.dram_tensor("LT", [n_prev, rpc], adt, kind="ExternalInput")
    disp = nc.dram_tensor("disp", [n, 1], F32, kind="ExternalInput")
    disbc = nc.dram_tensor("disbc", [128, rpc], F32, kind="ExternalInput")
    xpT = nc.dram_tensor("xpT", [C, n], F32R, kind="ExternalInput")
    Wd = nc.dram_tensor("Wd", [C, C], F32R, kind="ExternalInput")
    h = nc.dram_tensor("h", [rpc, C], F32, kind="ExternalInput")
    xo = nc.dram_tensor("xo", [rpc, C], F32, kind="ExternalOutput")
    if want_nt_out:
        nt_out = nc.dram_tensor("nt_out", [128, (n // 128) * rpc], BF16,
                                kind="ExternalOutput")
    with tile.TileContext(nc) as tc:
        ctx = ExitStack()
        sb = ctx.enter_context(tc.tile_pool(name="sb", bufs=1))
        ps = ctx.enter_context(tc.tile_pool(name="ps", bufs=8, space="PSUM"))
        R_sb = _load(nc, sb, R, "R")
        LT_sb = _load(nc, sb, LT, "LT")
        disp_sb = _load(nc, sb, disp, "disp")
        disbc_sb = sb.tile([128, rpc], F32, tag="disbc")
        nc.sync.dma_start(disbc_sb[:], disbc.ap())
        xpT_sb = _load(nc, sb, xpT, "xpT")
        Wd_sb = _load(nc, sb, Wd, "Wd")
        h_sb = _load(nc, sb, h, "h")
        nt_sb = sb.tile([128, n // 128, rpc], F32R, tag="nt")

        # M^T col-block -> scale rows by dis[j] (per-partition) and cols by
        # dis[i] (broadcast tile). Diag is NOT zeroed here; its effect on the
        # same-launch GCN is folded into `h`, and the host fixes nt_out.
        if want_nt_out:
            ntbf_sb = sb.tile([128, n // 128, rpc], BF16, tag="ntbf")

        def scale_nt(mo, p):
            nc.any.tensor_scalar_mul(nt_sb[:, mo, :], p,
                                     disp_sb[:, mo, :])
            nc.vector.tensor_tensor(nt_sb[:, mo, :], nt_sb[:, mo, :],
                                    disbc_sb[:], mybir.AluOpType.mult)
            if want_nt_out:
                nc.any.tensor_copy(ntbf_sb[:, mo, :], nt_sb[:, mo, :])

        _mm_block(nc, ps, [(R_sb, LT_sb, n_prev // 128)], n, rpc, scale_nt)
        if want_nt_out:
            nc.sync.dma_start(
                nt_out.ap().rearrange("p (o f) -> p o f", f=rpc), ntbf_sb[:])
        y_sb = sb.tile([128, n // 128, C], F32R, tag="y")
        _mm_block(nc, ps, [(xpT_sb, Wd_sb, C // 128)], n, C,
                  lambda mo, p: nc.any.tensor_copy(y_sb[:, mo, :], p),
                  tagp="ps")
        n_mo = (rpc + 127) // 128
        xo_sb = sb.tile([128, n_mo, C], F32, tag="xo")

        def fin(mo, p):
            nc.vector.tensor_tensor(xo_sb[:p.shape[0], mo, :], p,
                                    h_sb[:p.shape[0], mo, :],
                                    mybir.AluOpType.add)
            if want_relu:
                nc.vector.tensor_scalar_max(xo_sb[:p.shape[0], mo, :],
                                            xo_sb[:p.shape[0], mo, :], 0.0)

        _mm_block(nc, ps, [(nt_sb, y_sb, n // 128)], rpc, C, fin)
        if rpc >= 128:
            nc.sync.dma_start(_r3(xo.ap()), xo_sb[:])
        else:
            nc.sync.dma_start(xo.ap(), xo_sb[:rpc, 0, :])
        ctx.close()
    return _finish(nc)


# ------------------------------------------------------------------ K4b
def build_k4b():
    nc = _new_nc()
    NT2 = nc.dram_tensor("NT2", [1024, 1024], BF16, kind="ExternalInput")
    xa1w = nc.dram_tensor("xa1w", [1024, C], BF16, kind="ExternalInput")
    NT1b = nc.dram_tensor("NT1b", [2048, 256], BF16, kind="ExternalInput")
    Q1b = nc.dram_tensor("Q1b", [1024, 256], BF16, kind="ExternalInput")
    xd0 = nc.dram_tensor("xd0", [2048, C], BF16, kind="ExternalInput")
    Wu1 = nc.dram_tensor("Wu1", [C, C], BF16, kind="ExternalInput")
    ident = nc.dram_tensor("ident", [128, 128], BF16, kind="ExternalInput")
    xo = nc.dram_tensor("xo", [256, C], F32, kind="ExternalOutput")
    with tile.TileContext(nc) as tc:
        ctx = ExitStack()
        sb = ctx.enter_context(tc.tile_pool(name="sb", bufs=1))
        ps = ctx.enter_context(tc.tile_pool(name="ps", bufs=4, space="PSUM"))
        NT2_sb = _load(nc, sb, NT2, "NT2")
        xa1w_sb = _load(nc, sb, xa1w, "xa1w")
        NT1b_sb = _load(nc, sb, NT1b, "NT1b")
        Q1b_sb = _load(nc, sb, Q1b, "Q1b")
        xd0_sb = _load(nc, sb, xd0, "xd0")
        Wu1_sb = _load(nc, sb, Wu1, "Wu1")
        id_sb = _load(nc, sb, ident, "ident")
        # gcn2up (replicated): xU1 = relu(N2 @ (xa1 @ Wu0)), Wu0 folded
        # on host via associativity
        xU1 = sb.tile([128, 8, C], BF16, tag="xU1")

        def relu_to(dst):
            def f(mo, p):
                nc.vector.tensor_scalar_max(dst[:, mo, :], p, 0.0)
            return f

        _mm_block(nc, ps, [(NT2_sb, xa1w_sb, 8)], 1024, C, relu_to(xU1))
        # gcn1up sharded: v2_rows = N1[rows]@xd0 + N1[rows,perm1]@xU1
        v2 = sb.tile([128, 2, C], BF16, tag="v2")
        _mm_block(nc, ps, [(NT1b_sb, xd0_sb, 16), (Q1b_sb, xU1, 8)], 256, C,
                  lambda mo, p: nc.any.tensor_copy(v2[:, mo, :], p))
        v2T = _transpose_block(nc, sb, ps, id_sb[:, 0, :], v2, 2, "v2T")
        xo_sb = sb.tile([128, 2, C], F32, tag="xo")
        _mm_block(nc, ps, [(v2T, Wu1_sb, 2)], 256, C, relu_to(xo_sb))
        nc.sync.dma_start(_r3(xo.ap()), xo_sb[:])
        ctx.close()
    return _finish(nc)


# ------------------------------------------------------------------ K4c
def build_k4c():
    """out_rows = P0^T @ (x0@Wf) + Q0^T @ (xU2@Wf); dis folded into P0/Q0,
    Wf folded on host (matrix associativity), so no transposes remain."""
    nc = _new_nc()
    P0 = nc.dram_tensor("P0", [4096, 512], BF16, kind="ExternalInput")
    x0w = nc.dram_tensor("x0w", [4096, C], BF16, kind="ExternalInput")
    Q0 = nc.dram_tensor("Q0", [2048, 512], BF16, kind="ExternalInput")
    xU2w = nc.dram_tensor("xU2w", [2048, C], BF16, kind="ExternalInput")
    xo = nc.dram_tensor("xo", [512, C], F32, kind="ExternalOutput")
    with tile.TileContext(nc) as tc:
        ctx = ExitStack()
        sb = ctx.enter_context(tc.tile_pool(name="sb", bufs=1))
        ps = ctx.enter_context(tc.tile_pool(name="ps", bufs=8, space="PSUM"))
        P0_sb = _load(nc, sb, P0, "P0")
        x0w_sb = _load(nc, sb, x0w, "x0w")
        Q0_sb = _load(nc, sb, Q0, "Q0")
        xU2w_sb = _load(nc, sb, xU2w, "xU2w")
        xo_sb = sb.tile([128, 4, C], F32, tag="xo")
        _mm_block(nc, ps, [(P0_sb, x0w_sb, 32), (Q0_sb, xU2w_sb, 16)], 512,
                  C, lambda mo, p: nc.any.tensor_copy(xo_sb[:, mo, :], p))
        nc.sync.dma_start(_r3(xo.ap()), xo_sb[:])
        ctx.close()
    return _finish(nc)


# =================================================================== host
def _mk_dis(deg):
    return (1.0 / np.sqrt(np.maximum(deg, 1e-12))).astype(np.float32)


def kernel(x, edge_index, W_init, b_init, W_down, b_down, p_pool,
           W_up, b_up, W_final, b_final):
    x = np.asarray(x, np.float32)
    N = x.shape[0]
    rpc0 = N // NCORES
    ident = np.eye(128, dtype=np.float32)

    A0 = np.zeros((N, N), np.float32)
    np.add.at(A0, (np.asarray(edge_index[0]), np.asarray(edge_index[1])), 1.0)
    dis0 = _mk_dis(A0.sum(1) + 2.0)
    N0 = (dis0[:, None] * A0 * dis0[None, :]).astype(np.float32)
    N0[np.arange(N), np.arange(N)] += 2.0 * dis0 * dis0

    # ---- K1
    nc1 = build_k1(N, rpc0)
    xT = np.ascontiguousarray(x.T)
    maps = [{"xT": xT, "Wi": np.asarray(W_init, np.float32),
             "NT0": np.ascontiguousarray(N0[c * rpc0:(c + 1) * rpc0, :].T)}
            for c in range(NCORES)]
    res = _run(nc1, maps)
    x0 = np.concatenate([r["xo"] for r in res], 0)

    # ---- down levels
    xs = [x0]
    dis_l = [dis0]
    NT_blocks = []      # per level: list of per-core (D A D)^T blocks
    perms = []
    Acur_Bh = A0 + np.eye(N, dtype=np.float32)   # Bh of current level
    xcur = x0
    n = N
    level_dt = [FP8, FP8, BF16]
    for lev in range(3):
        p = np.asarray(p_pool[lev], np.float32)
        score = (xcur @ p) / np.linalg.norm(p)
        k = n // 2
        perm = np.argsort(-score, kind="stable")[:k]
        sv = score[perm]
        perms.append(perm)
        L = Acur_Bh[perm, :]
        R = Acur_Bh[:, perm]
        # host-side deg of pooled+augmented graph (without forming M)
        r = R.sum(1, dtype=np.float64)
        deg = (L @ r.astype(np.float32)).astype(np.float64) \
            - np.einsum('ak,ka->a', L, R, optimize=True) + 2.0
        dis = _mk_dis(deg.astype(np.float32))
        dis_l.append(dis)
        xp = (xcur[perm] * np.tanh(sv)[:, None]).astype(np.float32)
        Wd = np.asarray(W_down[lev], np.float32)
        y_full = xp @ Wd
        adt = level_dt[lev]
        npdt = NP_OF[adt]
        lim = 16 if adt == FP8 else 256
        assert Acur_Bh.max() <= lim, (lev, Acur_Bh.max())
        rpc = k // NCORES
        nc = build_level(n, k, rpc, adt, want_nt_out=(lev < 2))
        diagM = np.einsum('ak,ka->a', L, R, optimize=True)
        maps = []
        blocks = []
        for c in range(NCORES):
            sl = slice(c * rpc, (c + 1) * rpc)
            ig = np.arange(c * rpc, (c + 1) * rpc)
            # +2I term and removal of the spurious diag (dis^2*M_ii) in one
            hc = (dis[ig][:, None] ** 2 * (2.0 - diagM[sl][:, None])
                  * y_full[sl]).astype(np.float32)
            maps.append({
                "R": R.astype(npdt),
                "LT": np.ascontiguousarray(L[sl].T).astype(npdt),
                "disp": dis[:, None].astype(np.float32),
                "disbc": np.broadcast_to(dis[ig][None, :],
                                         (128, rpc)).copy(),
                "xpT": np.ascontiguousarray(xp.T),
                "Wd": Wd, "h": hc})
        res = _run(nc, maps)
        xcur = np.concatenate([r["xo"] for r in res], 0)
        if lev < 2:
            blocks = []
            for c in range(NCORES):
                KT = k // 128
                b = (res[c]["nt_out"].astype(np.float32)
                     .reshape(128, KT, rpc).transpose(1, 0, 2)
                     .reshape(k, rpc))
                ig = np.arange(c * rpc, (c + 1) * rpc)
                b[ig, np.arange(rpc)] = 0.0       # drop dis^2*M_ii diag
                blocks.append(b)
            NT_blocks.append(blocks)
            NTfull = np.concatenate(blocks, 1)     # = (D A D)^T, diag 0
            Anext = np.rint(NTfull.T / (dis[:, None] * dis[None, :]))
            Anext = Anext.astype(np.float32)
            Acur_Bh = Anext + np.eye(k, dtype=np.float32)
            xs.append(xcur)
        n = k

    x_d2 = xcur                                   # [512, C]
    x_d0, x_d1 = xs[1], xs[2]
    dis1, dis2 = dis_l[1], dis_l[2]

    # host scatter for deepest unpool: xa1 = x_d1 + scatter(perm2, x_d2)
    up = np.zeros_like(x_d1)
    up[perms[2]] = x_d2
    xa1 = (x_d1 + up).astype(np.float32)

    # N matrices with +2I diag restored
    NT2full = np.concatenate(NT_blocks[1], 1)
    NT2full[np.arange(1024), np.arange(1024)] += 2.0 * dis2 * dis2
    N1T_withI = np.concatenate(NT_blocks[0], 1)
    N1T_withI[np.arange(2048), np.arange(2048)] += 2.0 * dis1 * dis1
    N1full = N1T_withI.T

    # ---- K4b
    nc4b = build_k4b()
    rpc1 = 2048 // NCORES
    maps = []
    for c in range(NCORES):
        sl = slice(c * rpc1, (c + 1) * rpc1)
        bf = ml_dtypes.bfloat16
        xa1w = (xa1 @ np.asarray(W_up[0], np.float32)).astype(np.float32)
        maps.append({
            "NT2": NT2full.astype(bf), "xa1w": xa1w.astype(bf),
            "NT1b": np.ascontiguousarray(N1T_withI[:, sl]).astype(bf),
            "Q1b": np.ascontiguousarray(N1full[sl][:, perms[1]].T).astype(bf),
            "xd0": x_d0.astype(bf),
            "Wu1": np.asarray(W_up[1], np.float32).astype(bf),
            "ident": ident.astype(bf)})
    res = _run(nc4b, maps)
    xU2 = np.concatenate([r["xo"] for r in res], 0)    # [2048, C]

    # ---- K4c
    nc4c = build_k4c()
    Wf = np.asarray(W_final, np.float32)
    x0w = (x0 @ Wf).astype(np.float32)
    xU2w = (xU2 @ Wf).astype(np.float32)
    maps = []
    for c in range(NCORES):
        sl = slice(c * rpc0, (c + 1) * rpc0)
        bf = ml_dtypes.bfloat16
        maps.append({
            "P0": np.ascontiguousarray(N0[sl].T).astype(bf),
            "x0w": x0w.astype(bf),
            "Q0": np.ascontiguousarray(N0[sl][:, perms[0]].T).astype(bf),
            "xU2w": xU2w.astype(bf)})
    res = _run(nc4c, maps)
    out = np.concatenate([r["xo"] for r in res], 0)
    return out.astype(np.float32)



# revision 7
# speedup vs baseline: 1.3522x; 1.3522x over previous
"""GraphUNet (N=4096, E=65536, C=256, depth 3, ratio 0.5) on 8 trn2 NeuronCores.

Row-sharded SPMD pipeline, 6 launches. Device does the O(n^2 C) message
passing (N @ Z SpMMs) and the dense A@A augmentations; host does top-k,
gathers, O(nC^2) feature transforms / weight folds, and O(n^2)
element-wise adjacency prep between launches.

Traffic-minimized layout: every DRAM tensor is shipped partition-major
([128, K/128, F]) so each DMA descriptor moves >=512B contiguous runs at
full 360 GB/s model bandwidth.

  K1   init GCN:   x0[sl] = dis0*(A0[sl] @ (zhi+zlo)) + h, where A0 ships
                   as exact small-int fp8 (DoubleRow, 2x PE), z = dis0*(x@Wi)
                   ships as an fp8 hi+lo split (~0.1% residual), and h folds
                   the +2*dis^2 diagonal term and bias on host.
  K2-K4a levels:   M^T col-block = R^T @ L[sl]^T chain in fp8 (exact ints;
                   bf16 at level 3 where entries exceed 16), shipped back
                   raw (integers, bf16-exact) for host assembly; the down
                   GCN reuses the in-SBUF M^T block as lhsT against
                   host-prescaled yp = dis*(xp@Wd), with diag/bias folded
                   into a bf16 h and the dis row-scale applied per-psum.
  K4b  up GCNs:    xU1 = relu(N2 @ z2 + b) replicated (z2 host-folds the
                   deepest unpool scatter), then xU2[sl] = relu((N1[sl]@xd1
                   + N1[sl][:,perm1]@xU1) @ Wu1 + b) with one on-device
                   transpose; biases enter the psum via a rank-1 ones chain.
  K4c  final GCN:  identical program to K1 on zf = (x0 + scatter(perm0,
                   xU2)) @ Wf -- the unpool scatter and Wf fold on host
                   collapse the two chains of the reference into one.

Precision: adjacency chains exact (small ints in fp8/bf16); features bf16
(fp8 hi+lo where they feed the big A-chains); measured end-to-end rel err
~7e-3 vs the f32 reference.
"""

import numpy as np
import ml_dtypes

from contextlib import ExitStack

import concourse.bass as bass
import concourse.mybir as mybir
import concourse.tile as tile
from concourse import bacc
from concourse.bass_utils import run_bass_kernel_spmd

NCORES = 8
C = 256
F32 = mybir.dt.float32
F32R = mybir.dt.float32r
BF16 = mybir.dt.bfloat16
FP8 = mybir.dt.float8e4

NP_OF = {F32: np.float32, F32R: np.float32,
         BF16: ml_dtypes.bfloat16, FP8: ml_dtypes.float8_e4m3fn}

_TRACE = {"on": False, "results": [], "ncs": []}
DR = mybir.MatmulPerfMode.DoubleRow


def _new_nc():
    return bacc.Bacc("TRN2", target_bir_lowering=False, debug=False,
                     num_devices=NCORES)


def _finish(nc):
    nc.compile()
    _TRACE["ncs"].append(nc)
    return nc


def _run(nc, in_maps):
    res = run_bass_kernel_spmd(nc, in_maps, list(range(NCORES)),
                               trace=_TRACE["on"])
    if _TRACE["on"]:
        _TRACE["results"].append(res)
    return res.results


def _dma_kt(nc, t, dram, k0, k1, f0=None, f1=None):
    if f0 is None:
        nc.sync.dma_start(t[:, k0:k1, :], dram.ap()[:, k0:k1, :])
    else:
        nc.sync.dma_start(t[:, k0:k1, f0:f1], dram.ap()[:, k0:k1, f0:f1])


# ------------------------------------------------------------ K1 / K4c
def build_diag():
    """x[sl] = dis[sl] * (A0[sl] @ (zhi + zlo)) + h.   [4096 -> 512/core]"""
    n, rpc, KT, MO = 4096, 512, 32, 4
    nc = _new_nc()
    AT = nc.dram_tensor("AT", [128, KT, rpc], FP8, kind="ExternalInput")
    ZH = nc.dram_tensor("ZH", [128, KT, C], FP8, kind="ExternalInput")
    ZL = nc.dram_tensor("ZL", [128, KT, C], FP8, kind="ExternalInput")
    DISP = nc.dram_tensor("DISP", [128, MO], F32, kind="ExternalInput")
    H = nc.dram_tensor("H", [128, MO, C], BF16, kind="ExternalInput")
    XO = nc.dram_tensor("XO", [128, MO, C], F32, kind="ExternalOutput")
    with tile.TileContext(nc) as tc:
        ctx = ExitStack()
        sb = ctx.enter_context(tc.tile_pool(name="sb", bufs=1))
        ps = ctx.enter_context(tc.tile_pool(name="ps", bufs=1, space="PSUM"))
        zh_sb = sb.tile([128, KT, C], FP8, tag="zh")
        zl_sb = sb.tile([128, KT, C], FP8, tag="zl")
        disp_sb = sb.tile([128, MO], F32, tag="disp")
        h_sb = sb.tile([128, MO, C], BF16, tag="h")
        at_sb = sb.tile([128, KT, rpc], FP8, tag="at")
        for k0 in range(0, KT, 16):
            _dma_kt(nc, zh_sb, ZH, k0, k0 + 16)
        for k0 in range(0, KT, 16):
            _dma_kt(nc, zl_sb, ZL, k0, k0 + 16)
        nc.sync.dma_start(disp_sb[:], DISP.ap())
        nc.sync.dma_start(h_sb[:], H.ap())
        for k0 in range(0, KT, 8):
            _dma_kt(nc, at_sb, AT, k0, k0 + 8)
        pss = [ps.tile([128, C], F32, tag=f"p{m}", name=f"p{m}")
               for m in range(MO)]
        xo_sb = sb.tile([128, MO, C], F32, tag="xo")
        for kp in range(KT // 2):
            for mo in range(MO):
                for ci, ch in enumerate((zh_sb, zl_sb)):
                    nc.tensor.matmul(
                        pss[mo][:], at_sb[:, 2 * kp:2 * kp + 2,
                                          mo * 128:(mo + 1) * 128],
                        ch[:, 2 * kp:2 * kp + 2, :],
                        start=(kp == 0 and ci == 0),
                        stop=(kp == KT // 2 - 1 and ci == 1),
                        perf_mode=DR)
        for mo in range(MO):
            nc.any.tensor_scalar_mul(xo_sb[:, mo, :], pss[mo][:],
                                     disp_sb[:, mo:mo + 1])
            nc.vector.tensor_tensor(xo_sb[:, mo, :], xo_sb[:, mo, :],
                                    h_sb[:, mo, :], mybir.AluOpType.add)
        nc.sync.dma_start(XO.ap(), xo_sb[:])
        ctx.close()
    return _finish(nc)


# ----------------------------------------------------- K2 / K3 / K4a
def build_level(npv, n, rpc, adt, ship):
    """M^T col-block = R^T @ L[sl]^T (exact ints), then
    x[sl] = relu(dis[sl] * (M[sl] @ yp) + h).  Ships raw M^T if `ship`."""
    KTp, KT = npv // 128, n // 128
    mo2 = (rpc + 127) // 128
    msz2 = min(128, rpc)
    mdt = BF16 if ship else F32R
    ydt = BF16 if ship else F32R
    nc = _new_nc()
    R = nc.dram_tensor("R", [128, KTp, n], adt, kind="ExternalInput")
    LT = nc.dram_tensor("LT", [128, KTp, rpc], adt, kind="ExternalInput")
    YP = nc.dram_tensor("YP", [128, KT, C], ydt, kind="ExternalInput")
    DISP = nc.dram_tensor("DISP", [128, mo2], F32, kind="ExternalInput")
    hsh = [128, mo2, C] if rpc >= 128 else [rpc, C]
    H = nc.dram_tensor("H", hsh, BF16, kind="ExternalInput")
    if ship:
        MT = nc.dram_tensor("MT", [128, KT, rpc], BF16,
                            kind="ExternalOutput")
    XO = nc.dram_tensor("XO", [128, mo2, C] if rpc >= 128 else [rpc, C],
                        F32, kind="ExternalOutput")
    groups = [list(range(g, min(g + 8, KT))) for g in range(0, KT, 8)]
    with tile.TileContext(nc) as tc:
        ctx = ExitStack()
        sb = ctx.enter_context(tc.tile_pool(name="sb", bufs=1))
        ps = ctx.enter_context(tc.tile_pool(name="ps", bufs=1, space="PSUM"))
        lt_sb = sb.tile([128, KTp, rpc], adt, tag="lt")
        for k0 in range(0, KTp, 8):
            _dma_kt(nc, lt_sb, LT, k0, min(KTp, k0 + 8))
        disp_sb = sb.tile([128, mo2], F32, tag="disp")
        nc.sync.dma_start(disp_sb[:], DISP.ap())
        h_sb = sb.tile([128, mo2, C], BF16, tag="h")
        if rpc >= 128:
            nc.sync.dma_start(h_sb[:], H.ap())
        else:
            nc.sync.dma_start(h_sb[:rpc, 0, :], H.ap())
        r_sb = sb.tile([128, KTp, n], adt, tag="r")
        mt_sb = sb.tile([128, KT, rpc], mdt, tag="mt")
        yp_sb = sb.tile([128, KT, C], ydt, tag="yp")
        use_dr = adt == FP8
        for gi, mos in enumerate(groups):
            f0, f1 = mos[0] * 128, (mos[-1] + 1) * 128
            for k0 in range(0, KTp, 8):
                _dma_kt(nc, r_sb, R, k0, min(KTp, k0 + 8), f0, f1)
            pss = {m: ps.tile([128, rpc], F32, tag=f"p{m % 8}",
                              name=f"p{m}") for m in mos}
            if use_dr:
                for kp in range(KTp // 2):
                    for mo in mos:
                        nc.tensor.matmul(
                            pss[mo][:],
                            r_sb[:, 2 * kp:2 * kp + 2,
                                 mo * 128:(mo + 1) * 128],
                            lt_sb[:, 2 * kp:2 * kp + 2, :],
                            start=(kp == 0), stop=(kp == KTp // 2 - 1),
                            perf_mode=DR)
            else:
                for kt in range(KTp):
                    for mo in mos:
                        nc.tensor.matmul(
                            pss[mo][:],
                            r_sb[:, kt, mo * 128:(mo + 1) * 128],
                            lt_sb[:, kt, :],
                            start=(kt == 0), stop=(kt == KTp - 1))
            for mo in mos:
                nc.any.tensor_copy(mt_sb[:, mo, :], pss[mo][:])
            if ship:
                nc.sync.dma_start(MT.ap()[:, mos[0]:mos[-1] + 1, :],
                                  mt_sb[:, mos[0]:mos[-1] + 1, :])
        for k0 in range(0, KT, 8):
            _dma_kt(nc, yp_sb, YP, k0, min(KT, k0 + 8))
        xo_sb = sb.tile([128, mo2, C], F32, tag="xo")
        for mo in range(mo2):
            msz = min(128, rpc - mo * 128)
            ps2 = ps.tile([128, C], F32, tag=f"p{mo}", name="pg")
            for kt in range(KT):
                nc.tensor.matmul(
                    ps2[:msz, :],
                    mt_sb[:, kt, mo * 128:mo * 128 + msz],
                    yp_sb[:, kt, :], start=(kt == 0), stop=(kt == KT - 1))
            nc.any.tensor_scalar_mul(xo_sb[:msz, mo, :], ps2[:msz, :],
                                     disp_sb[:msz, mo:mo + 1])
            nc.vector.tensor_tensor(xo_sb[:msz, mo, :], xo_sb[:msz, mo, :],
                                    h_sb[:msz, mo, :], mybir.AluOpType.add)
            nc.vector.tensor_scalar_max(xo_sb[:msz, mo, :],
                                        xo_sb[:msz, mo, :], 0.0)
        if rpc >= 128:
            nc.sync.dma_start(XO.ap(), xo_sb[:])
        else:
            nc.sync.dma_start(XO.ap(), xo_sb[:rpc, 0, :])
        ctx.close()
    return _finish(nc)


# ------------------------------------------------------------------ K4b
def build_k4b():
    """xU1 = relu(N2 @ z2 + b0) replicated;
    xU2[sl] = relu((N1[sl]@xd1 + N1[sl][:,perm1]@xU1) @ Wu1 + b1)."""
    nc = _new_nc()
    NT2 = nc.dram_tensor("NT2", [128, 8, 1024], BF16, kind="ExternalInput")
    Z2 = nc.dram_tensor("Z2", [128, 8, C], BF16, kind="ExternalInput")
    NT1B = nc.dram_tensor("NT1B", [128, 16, C], BF16, kind="ExternalInput")
    XD1 = nc.dram_tensor("XD1", [128, 16, C], BF16, kind="ExternalInput")
    Q1B = nc.dram_tensor("Q1B", [128, 8, C], BF16, kind="ExternalInput")
    WU1 = nc.dram_tensor("WU1", [128, 2, C], BF16, kind="ExternalInput")
    IDT = nc.dram_tensor("IDT", [128, 128], BF16, kind="ExternalInput")
    ONES = nc.dram_tensor("ONES", [1, 128], BF16, kind="ExternalInput")
    B0 = nc.dram_tensor("B0", [1, C], BF16, kind="ExternalInput")
    B1 = nc.dram_tensor("B1", [1, C], BF16, kind="ExternalInput")
    XO = nc.dram_tensor("XO", [128, 2, C], F32, kind="ExternalOutput")
    with tile.TileContext(nc) as tc:
        ctx = ExitStack()
        sb = ctx.enter_context(tc.tile_pool(name="sb", bufs=1))
        ps = ctx.enter_context(tc.tile_pool(name="ps", bufs=1, space="PSUM"))
        z2_sb = sb.tile([128, 8, C], BF16, tag="z2")
        nc.sync.dma_start(z2_sb[:], Z2.ap())
        ones_sb = sb.tile([128, 128], BF16, tag="ones")
        nc.sync.dma_start(ones_sb[:1, :], ONES.ap())
        b0_sb = sb.tile([128, C], BF16, tag="b0")
        nc.sync.dma_start(b0_sb[:1, :], B0.ap())
        b1_sb = sb.tile([128, C], BF16, tag="b1")
        nc.sync.dma_start(b1_sb[:1, :], B1.ap())
        idt_sb = sb.tile([128, 128], BF16, tag="idt")
        nc.sync.dma_start(idt_sb[:], IDT.ap())
        wu1_sb = sb.tile([128, 2, C], BF16, tag="wu1")
        nc.sync.dma_start(wu1_sb[:], WU1.ap())
        nt2_sb = sb.tile([128, 8, 1024], BF16, tag="nt2")
        for k0 in range(0, 8, 2):
            _dma_kt(nc, nt2_sb, NT2, k0, k0 + 2)
        xd1_sb = sb.tile([128, 16, C], BF16, tag="xd1")
        for k0 in range(0, 16, 8):
            _dma_kt(nc, xd1_sb, XD1, k0, k0 + 8)
        nt1b_sb = sb.tile([128, 16, C], BF16, tag="nt1b")
        for k0 in range(0, 16, 8):
            _dma_kt(nc, nt1b_sb, NT1B, k0, k0 + 8)
        q1b_sb = sb.tile([128, 8, C], BF16, tag="q1b")
        nc.sync.dma_start(q1b_sb[:], Q1B.ap())

        xu1_sb = sb.tile([128, 8, C], BF16, tag="xu1")
        pss = [ps.tile([128, C], F32, tag=f"p{m}", name=f"p{m}")
               for m in range(8)]
        for kt in range(8):
            for mo in range(8):
                nc.tensor.matmul(
                    pss[mo][:], nt2_sb[:, kt, mo * 128:(mo + 1) * 128],
                    z2_sb[:, kt, :], start=(kt == 0), stop=False)
        for mo in range(8):
            nc.tensor.matmul(pss[mo][:], ones_sb[:1, :], b0_sb[:1, :],
                             start=False, stop=True)
            nc.vector.tensor_scalar_max(xu1_sb[:, mo, :], pss[mo][:], 0.0)

        v2_sb = sb.tile([128, 2, C], BF16, tag="v2")
        for mo in range(2):
            ps2 = ps.tile([128, C], F32, tag=f"p{mo}", name="pv")
            for kt in range(16):
                nc.tensor.matmul(
                    ps2[:], nt1b_sb[:, kt, mo * 128:(mo + 1) * 128],
                    xd1_sb[:, kt, :], start=(kt == 0), stop=False)
            for kt in range(8):
                nc.tensor.matmul(
                    ps2[:], q1b_sb[:, kt, mo * 128:(mo + 1) * 128],
                    xu1_sb[:, kt, :], start=False, stop=(kt == 7))
            nc.any.tensor_copy(v2_sb[:, mo, :], ps2[:])
        v2t_sb = sb.tile([128, 2, C], BF16, tag="v2t")
        for mo in range(2):
            for cc in range(2):
                pst = ps.tile([128, 128], BF16, tag=f"p{2 + mo * 2 + cc}", name="pt")
                nc.tensor.transpose(pst[:],
                                    v2_sb[:, mo, cc * 128:(cc + 1) * 128],
                                    idt_sb[:])
                nc.any.tensor_copy(v2t_sb[:, cc, mo * 128:(mo + 1) * 128],
                                   pst[:])
        xo_sb = sb.tile([128, 2, C], F32, tag="xo")
        for mo in range(2):
            ps3 = ps.tile([128, C], F32, tag=f"p{6 + mo}", name="pw")
            for kt in range(2):
                nc.tensor.matmul(
                    ps3[:], v2t_sb[:, kt, mo * 128:(mo + 1) * 128],
                    wu1_sb[:, kt, :], start=(kt == 0), stop=False)
            nc.tensor.matmul(ps3[:], ones_sb[:1, :], b1_sb[:1, :],
                             start=False, stop=True)
            nc.vector.tensor_scalar_max(xo_sb[:, mo, :], ps3[:], 0.0)
        nc.sync.dma_start(XO.ap(), xo_sb[:])
        ctx.close()
    return _finish(nc)


# =================================================================== host
F8NP = ml_dtypes.float8_e4m3fn
BFNP = ml_dtypes.bfloat16


def _pm(a, dt):
    """[K, F] row-major -> partition-major [128, K//128, F]."""
    K, F = a.shape
    return np.ascontiguousarray(
        a.reshape(K // 128, 128, F).transpose(1, 0, 2)).astype(dt)


def _unpm(b):
    """[128, KT, F] -> [K, F]."""
    p, kt, f = b.shape
    return np.asarray(b, np.float32).transpose(1, 0, 2).reshape(kt * p, f)


def _pmv(v):
    """[K] -> [128, K//128] partition-major (padded to 128 rows)."""
    k = v.shape[0]
    if k < 128:
        v = np.pad(v, (0, 128 - k))
        k = 128
    return np.ascontiguousarray(
        v.reshape(k // 128, 128).T).astype(np.float32)


def _mk_dis(deg):
    return (1.0 / np.sqrt(np.maximum(deg, 1e-12))).astype(np.float32)


def _diag_inputs(A8T, z, dis, y, bvec, rpc):
    """Per-core in_maps for the K1/K4c program."""
    zhi = z.astype(F8NP)
    zlo = (z - zhi.astype(np.float32)).astype(F8NP)
    zhi_pm, zlo_pm = _pm(zhi, F8NP), _pm(zlo, F8NP)
    maps = []
    for c in range(NCORES):
        sl = slice(c * rpc, (c + 1) * rpc)
        h = (2.0 * dis[sl] ** 2)[:, None] * y[sl] + bvec
        maps.append({
            "AT": np.ascontiguousarray(A8T[:, :, sl]),
            "ZH": zhi_pm, "ZL": zlo_pm,
            "DISP": _pmv(dis[sl]),
            "H": _pm(h.astype(np.float32), BFNP),
            })
    return maps


def kernel(x, edge_index, W_init, b_init, W_down, b_down, p_pool,
           W_up, b_up, W_final, b_final):
    x = np.asarray(x, np.float32)
    N = x.shape[0]
    rpc0 = N // NCORES

    A0 = np.zeros((N, N), np.float32)
    np.add.at(A0, (np.asarray(edge_index[0]), np.asarray(edge_index[1])),
              1.0)
    dis0 = _mk_dis(A0.sum(1) + 2.0)
    y0 = x @ np.asarray(W_init, np.float32)

    # exact level-0 score via host matvec (init GCN is linear)
    p0 = np.asarray(p_pool[0], np.float32)
    u = y0 @ p0
    s0 = (dis0 * (A0 @ (dis0 * u)) + 2.0 * dis0 * dis0 * u) \
        / np.linalg.norm(p0)
    perm0 = np.argsort(-s0, kind="stable")[:N // 2]
    sv0 = s0[perm0]

    # ---- K1
    A8Tpm = _pm(np.ascontiguousarray(A0.T), F8NP)  # [128, 32, 4096]
    nc1 = build_diag()
    maps = _diag_inputs(A8Tpm, dis0[:, None] * y0, dis0, y0,
                        np.asarray(b_init, np.float32), rpc0)
    res = _run(nc1, maps)
    x0 = np.concatenate([_unpm(r["XO"]) for r in res], 0)

    # ---- down levels
    Bh = A0 + np.eye(N, dtype=np.float32)
    xcur, perm, sv = x0, perm0, sv0
    n = N
    Ms, dis_l, xs, perms = [], [dis0], [x0], []
    level_fp8 = [True, True, False]
    for lev in range(3):
        k = n // 2
        rpc = k // NCORES
        perms.append(perm)
        L = Bh[perm, :]
        R = Bh[:, perm]
        lim = 16 if level_fp8[lev] else 256
        assert Bh.max() <= lim, (lev, Bh.max())
        diagM = np.einsum('it,ti->i', L, R, optimize=True)
        deg = L @ R.sum(1) - diagM + 2.0
        dis = _mk_dis(deg)
        xp = xcur[perm] * np.tanh(sv)[:, None]
        y = xp @ np.asarray(W_down[lev], np.float32)
        adt = FP8 if level_fp8[lev] else BF16
        npdt = NP_OF[adt]
        ship = lev < 2
        ydt = BFNP if ship else np.float32
        nc = build_level(n, k, rpc, adt, ship)
        Rpm = _pm(R, npdt)
        yp_pm = _pm(dis[:, None] * y, ydt)
        maps = []
        for cc in range(NCORES):
            sl = slice(cc * rpc, (cc + 1) * rpc)
            h = (dis[sl] ** 2 * (2.0 - diagM[sl]))[:, None] * y[sl] \
                + np.asarray(b_down[lev], np.float32)
            if rpc >= 128:
                h_pm = _pm(h.astype(np.float32), BFNP)
            else:
                h_pm = np.ascontiguousarray(h.astype(BFNP))
            maps.append({
                "R": Rpm,
                "LT": _pm(np.ascontiguousarray(L[sl].T), npdt),
                "YP": yp_pm,
                "DISP": _pmv(dis[sl]),
                "H": h_pm,
                })
        res = _run(nc, maps)
        if rpc >= 128:
            xn = np.concatenate([_unpm(r["XO"]) for r in res], 0)
        else:
            xn = np.concatenate([np.asarray(r["XO"], np.float32)
                                 for r in res], 0)
        if ship:
            M = np.concatenate([_unpm(r["MT"]).T for r in res], 0)
            Ms.append(M)
            Bh = M.copy()
            np.fill_diagonal(Bh, 1.0)
        dis_l.append(dis)
        xs.append(xn)
        xcur, n = xn, k
        if lev < 2:
            pl = np.asarray(p_pool[lev + 1], np.float32)
            s = xn @ pl / np.linalg.norm(pl)
            perm = np.argsort(-s, kind="stable")[:k // 2]
            sv = s[perm]

    x_d1, x_d2, x_d3 = xs[1], xs[2], xs[3]
    dis1, dis2 = dis_l[1], dis_l[2]
    M1, M2 = Ms[0], Ms[1]
    perm1, perm2 = perms[1], perms[2]

    # ---- K4b
    N2 = M2.copy()
    np.fill_diagonal(N2, 2.0)
    N2 *= dis2[:, None] * dis2[None, :]
    N1 = M1.copy()
    np.fill_diagonal(N1, 2.0)
    N1 *= dis1[:, None] * dis1[None, :]
    up = np.zeros_like(x_d2)
    up[perm2] = x_d3
    z2 = (x_d2 + up) @ np.asarray(W_up[0], np.float32)
    nc4b = build_k4b()
    rpc1 = 2048 // NCORES
    nt2_pm = _pm(np.ascontiguousarray(N2.T), BFNP)
    z2_pm = _pm(z2, BFNP)
    xd1_pm = _pm(x_d1, BFNP)
    wu1_pm = _pm(np.asarray(W_up[1], np.float32), BFNP)
    idt = np.eye(128, dtype=np.float32).astype(BFNP)
    ones = np.ones((1, 128), BFNP)
    b0 = np.asarray(b_up[0], np.float32)[None, :].astype(BFNP)
    b1 = np.asarray(b_up[1], np.float32)[None, :].astype(BFNP)
    maps = []
    for cc in range(NCORES):
        sl = slice(cc * rpc1, (cc + 1) * rpc1)
        maps.append({
            "NT2": nt2_pm, "Z2": z2_pm,
            "NT1B": _pm(np.ascontiguousarray(N1[sl].T), BFNP),
            "XD1": xd1_pm,
            "Q1B": _pm(np.ascontiguousarray(N1[sl][:, perm1].T), BFNP),
            "WU1": wu1_pm, "IDT": idt, "ONES": ones, "B0": b0, "B1": b1,
            })
    res = _run(nc4b, maps)
    xU2 = np.concatenate([_unpm(r["XO"]) for r in res], 0)

    # ---- K4c
    upf = np.zeros_like(x0)
    upf[perm0] = xU2
    zf = (x0 + upf) @ np.asarray(W_final, np.float32)
    nc4c = build_diag()
    maps = _diag_inputs(A8Tpm, dis0[:, None] * zf, dis0, zf,
                        np.asarray(b_final, np.float32), rpc0)
    res = _run(nc4c, maps)
    out = np.concatenate([_unpm(r["XO"]) for r in res], 0)
    return out.astype(np.float32)


# revision 10
# speedup vs baseline: 1.3802x; 1.0207x over previous
"""GraphUNet (N=4096, E=65536, C=256, depth 3, ratio 0.5) on 8 trn2 NeuronCores.

Row-sharded SPMD pipeline, 6 launches. Device does the O(n^2 C) message
passing (N @ Z SpMMs) and the dense A@A augmentations; host does top-k,
gathers, O(nC^2) feature transforms / weight folds, and O(n^2)
element-wise adjacency prep between launches.

Layout/overlap notes: every DRAM tensor ships partition-major
([128, K/128, F]) so DMA descriptors move >=512B contiguous runs at the
full modeled 360 GB/s; input DMAs are emitted in consumption order with
the streamed operand chunk-interleaved so the PE starts ~4us in; output
DMAs are emitted last (the DMA queue is in-order, so a store's semaphore
wait would block later loads); diag/bias GCN corrections are folded into
the PSUM accumulation as rank-1 / diagonal matmul chains, leaving one
fused tensor_scalar per output block.

  K1   init GCN:   x0[sl] = dis0*(A0[sl] @ (zhi+zlo) + 2*dis0*y0[sl]
                   + (1/dis0) x b), A0 in exact small-int fp8 (DoubleRow),
                   z = dis0*(x@Wi) as an fp8 hi+lo split.
  K2-K4a levels:   M^T col-block = R^T @ L[sl]^T chain in fp8 (exact ints;
                   bf16 at level 3), shipped back raw (bf16-exact ints);
                   the down GCN reuses the in-SBUF M^T block as lhsT
                   against host-prescaled yp = dis*(xp@Wd), with the
                   diag/bias corrections in-chain and relu fused into the
                   dis row-scale consumer.
  K4b  up GCNs:    xU1 = relu(N2 @ z2 + b) replicated (z2 host-folds the
                   deepest unpool scatter), then xU2[sl] = relu((N1[sl]@xd1
                   + N1[sl][:,perm1]@xU1) @ Wu1 + b) with one on-device
                   transpose; biases enter the psum via rank-1 ones chains.
  K4c  final GCN:  identical program to K1 on zf = (x0 + scatter(perm0,
                   xU2)) @ Wf -- the unpool scatter and Wf fold on host
                   collapse the two chains of the reference into one.

Precision: adjacency chains exact; features bf16 (fp8 hi+lo where they
feed the big A-chains); measured end-to-end rel err ~7e-3 vs f32 ref.
"""

import numpy as np
import ml_dtypes

from contextlib import ExitStack

import concourse.bass as bass
import concourse.mybir as mybir
import concourse.tile as tile
from concourse import bacc
from concourse.bass_utils import run_bass_kernel_spmd

NCORES = 8
C = 256
F32 = mybir.dt.float32
F32R = mybir.dt.float32r
BF16 = mybir.dt.bfloat16
FP8 = mybir.dt.float8e4

NP_OF = {F32: np.float32, F32R: np.float32,
         BF16: ml_dtypes.bfloat16, FP8: ml_dtypes.float8_e4m3fn}

_TRACE = {"on": False, "results": [], "ncs": []}
DR = mybir.MatmulPerfMode.DoubleRow
MULT = mybir.AluOpType.mult
MAXOP = mybir.AluOpType.max


def _new_nc():
    return bacc.Bacc("TRN2", target_bir_lowering=False, debug=False,
                     num_devices=NCORES)


def _finish(nc):
    nc.compile()
    _TRACE["ncs"].append(nc)
    return nc


def _run(nc, in_maps):
    res = run_bass_kernel_spmd(nc, in_maps, list(range(NCORES)),
                               trace=_TRACE["on"])
    if _TRACE["on"]:
        _TRACE["results"].append(res)
    return res.results


def _ld(nc, t, dram, k0, k1, f0=None, f1=None):
    if f0 is None:
        nc.sync.dma_start(t[:, k0:k1, :], dram.ap()[:, k0:k1, :])
    else:
        nc.sync.dma_start(t[:, k0:k1, f0:f1], dram.ap()[:, k0:k1, f0:f1])


# ------------------------------------------------------------ K1 / K4c
def build_diag():
    """x[sl] = dis[sl]*(A0[sl] @ (zhi+zlo) + diag(2 dis[sl]) y[sl]
    + (1/dis[sl]) x b).   [4096 -> 512/core]"""
    n, rpc, KT, MO = 4096, 512, 32, 4
    nc = _new_nc()
    AT = nc.dram_tensor("AT", [128, KT, rpc], FP8, kind="ExternalInput")
    ZH = nc.dram_tensor("ZH", [128, KT, C], FP8, kind="ExternalInput")
    ZL = nc.dram_tensor("ZL", [128, KT, C], FP8, kind="ExternalInput")
    DISP = nc.dram_tensor("DISP", [128, MO], F32, kind="ExternalInput")
    DG = nc.dram_tensor("DG", [128, MO, 128], BF16, kind="ExternalInput")
    YS = nc.dram_tensor("YS", [128, MO, C], BF16, kind="ExternalInput")
    IV = nc.dram_tensor("IV", [1, rpc], BF16, kind="ExternalInput")
    BV = nc.dram_tensor("BV", [1, C], BF16, kind="ExternalInput")
    XO = nc.dram_tensor("XO", [128, MO, C], F32, kind="ExternalOutput")
    with tile.TileContext(nc) as tc:
        ctx = ExitStack()
        sb = ctx.enter_context(tc.tile_pool(name="sb", bufs=1))
        ps = ctx.enter_context(tc.tile_pool(name="ps", bufs=1, space="PSUM"))
        disp_sb = sb.tile([128, MO], F32, tag="disp")
        nc.sync.dma_start(disp_sb[:], DISP.ap())
        dg_sb = sb.tile([128, MO, 128], BF16, tag="dg")
        nc.sync.dma_start(dg_sb[:], DG.ap())
        ys_sb = sb.tile([128, MO, C], BF16, tag="ys")
        nc.sync.dma_start(ys_sb[:], YS.ap())
        iv_sb = sb.tile([128, rpc], BF16, tag="iv")
        nc.sync.dma_start(iv_sb[:1, :], IV.ap())
        bv_sb = sb.tile([128, C], BF16, tag="bv")
        nc.sync.dma_start(bv_sb[:1, :], BV.ap())
        zh_sb = sb.tile([128, KT, C], FP8, tag="zh")
        zl_sb = sb.tile([128, KT, C], FP8, tag="zl")
        at_sb = sb.tile([128, KT, rpc], FP8, tag="at")
        for k0 in range(0, KT, 8):
            _ld(nc, zh_sb, ZH, k0, k0 + 8)
            _ld(nc, zl_sb, ZL, k0, k0 + 8)
            _ld(nc, at_sb, AT, k0, k0 + 8)
        pss = [ps.tile([128, C], F32, tag=f"p{m}", name=f"p{m}")
               for m in range(MO)]
        xo_sb = sb.tile([128, MO, C], F32, tag="xo")
        for mo in range(MO):
            nc.tensor.matmul(pss[mo][:], dg_sb[:, mo, :], ys_sb[:, mo, :],
                             start=True, stop=False)
            nc.tensor.matmul(pss[mo][:],
                             iv_sb[:1, mo * 128:(mo + 1) * 128],
                             bv_sb[:1, :], start=False, stop=False)
        for kp in range(KT // 2):
            for mo in range(MO):
                for ci, ch in enumerate((zh_sb, zl_sb)):
                    nc.tensor.matmul(
                        pss[mo][:], at_sb[:, 2 * kp:2 * kp + 2,
                                          mo * 128:(mo + 1) * 128],
                        ch[:, 2 * kp:2 * kp + 2, :],
                        start=False,
                        stop=(kp == KT // 2 - 1 and ci == 1),
                        perf_mode=DR)
        for mo in range(MO):
            nc.any.tensor_scalar_mul(xo_sb[:, mo, :], pss[mo][:],
                                     disp_sb[:, mo:mo + 1])
        for mo in range(MO):
            nc.sync.dma_start(XO.ap()[:, mo, :], xo_sb[:, mo, :])
        ctx.close()
    return _finish(nc)


# ----------------------------------------------------- K2 / K3 / K4a
def build_level(npv, n, rpc, adt, ship):
    """M^T col-block = R^T @ L[sl]^T (exact ints), then
    x[sl] = relu(dis[sl] * (M[sl] @ yp + diag(2-diagM) yp[sl]
    + (1/dis) x b)).  Ships raw M^T if `ship`."""
    KTp, KT = npv // 128, n // 128
    mo2 = (rpc + 127) // 128
    mdt = BF16 if ship else F32R
    ydt = BF16 if ship else F32R
    nc = _new_nc()
    R = nc.dram_tensor("R", [128, KTp, n], adt, kind="ExternalInput")
    LT = nc.dram_tensor("LT", [128, KTp, rpc], adt, kind="ExternalInput")
    YP = nc.dram_tensor("YP", [128, KT, C], ydt, kind="ExternalInput")
    DISP = nc.dram_tensor("DISP", [128, mo2], F32, kind="ExternalInput")
    DG = nc.dram_tensor("DG", [128, mo2, 128], ydt, kind="ExternalInput")
    YS = nc.dram_tensor("YS", [128, mo2, C], ydt, kind="ExternalInput")
    IV = nc.dram_tensor("IV", [1, max(rpc, 128)], ydt,
                        kind="ExternalInput")
    BV = nc.dram_tensor("BV", [1, C], ydt, kind="ExternalInput")
    if ship:
        MT = nc.dram_tensor("MT", [128, KT, rpc], BF16,
                            kind="ExternalOutput")
    XO = nc.dram_tensor("XO", [128, mo2, C] if rpc >= 128 else [rpc, C],
                        F32, kind="ExternalOutput")
    groups = [list(range(g, min(g + 6, KT))) for g in range(0, KT, 6)]
    with tile.TileContext(nc) as tc:
        ctx = ExitStack()
        sb = ctx.enter_context(tc.tile_pool(name="sb", bufs=1))
        ps = ctx.enter_context(tc.tile_pool(name="ps", bufs=1, space="PSUM"))
        disp_sb = sb.tile([128, mo2], F32, tag="disp")
        nc.sync.dma_start(disp_sb[:], DISP.ap())
        dg_sb = sb.tile([128, mo2, 128], ydt, tag="dg")
        nc.sync.dma_start(dg_sb[:], DG.ap())
        ys_sb = sb.tile([128, mo2, C], ydt, tag="ys")
        nc.sync.dma_start(ys_sb[:], YS.ap())
        iv_sb = sb.tile([128, max(rpc, 128)], ydt, tag="iv")
        nc.sync.dma_start(iv_sb[:1, :], IV.ap())
        bv_sb = sb.tile([128, C], ydt, tag="bv")
        nc.sync.dma_start(bv_sb[:1, :], BV.ap())
        lt_sb = sb.tile([128, KTp, rpc], adt, tag="lt")
        for k0 in range(0, KTp, 8):
            _ld(nc, lt_sb, LT, k0, min(KTp, k0 + 8))
        r_sb = sb.tile([128, KTp, n], adt, tag="r")
        yp_sb = sb.tile([128, KT, C], ydt, tag="yp")
        for gi, mos in enumerate(groups):
            f0, f1 = mos[0] * 128, (mos[-1] + 1) * 128
            for k0 in range(0, KTp, 8):
                _ld(nc, r_sb, R, k0, min(KTp, k0 + 8), f0, f1)
            if gi == 0:
                for k0 in range(0, KT, 8):
                    _ld(nc, yp_sb, YP, k0, min(KT, k0 + 8))
        mt_sb = sb.tile([128, KT, rpc], mdt, tag="mt")
        xo_sb = sb.tile([128, mo2, C], F32, tag="xo")
        use_dr = adt == FP8
        gps = [ps.tile([128, C], F32, tag=f"g{m}", name=f"g{m}")
               for m in range(mo2)]
        msz2 = min(128, rpc)
        for m in range(mo2):
            nc.tensor.matmul(gps[m][:msz2, :], dg_sb[:msz2, m, :msz2],
                             ys_sb[:msz2, m, :], start=True, stop=False)
            nc.tensor.matmul(gps[m][:msz2, :],
                             iv_sb[:1, m * 128:m * 128 + msz2],
                             bv_sb[:1, :], start=False, stop=False)
        for gi, mos in enumerate(groups):
            pss = {m: ps.tile([128, rpc], F32, tag=f"p{m % 6}",
                              name=f"p{m}") for m in mos}
            if use_dr:
                for kp in range(KTp // 2):
                    for mo in mos:
                        nc.tensor.matmul(
                            pss[mo][:],
                            r_sb[:, 2 * kp:2 * kp + 2,
                                 mo * 128:(mo + 1) * 128],
                            lt_sb[:, 2 * kp:2 * kp + 2, :],
                            start=(kp == 0), stop=(kp == KTp // 2 - 1),
                            perf_mode=DR)
            else:
                for kt in range(KTp):
                    for mo in mos:
                        nc.tensor.matmul(
                            pss[mo][:],
                            r_sb[:, kt, mo * 128:(mo + 1) * 128],
                            lt_sb[:, kt, :],
                            start=(kt == 0), stop=(kt == KTp - 1))
            for mo in mos:
                nc.any.tensor_copy(mt_sb[:, mo, :], pss[mo][:])
            for m in range(mo2):
                for kt in mos:
                    nc.tensor.matmul(
                        gps[m][:msz2, :],
                        mt_sb[:, kt, m * 128:m * 128 + msz2],
                        yp_sb[:, kt, :], start=False, stop=(kt == KT - 1))
        for m in range(mo2):
            nc.vector.tensor_scalar(xo_sb[:msz2, m, :], gps[m][:msz2, :],
                                    disp_sb[:msz2, m:m + 1], 0.0,
                                    MULT, MAXOP)
        if ship:
            for gi, mos in enumerate(groups):
                nc.sync.dma_start(MT.ap()[:, mos[0]:mos[-1] + 1, :],
                                  mt_sb[:, mos[0]:mos[-1] + 1, :])
        if rpc >= 128:
            for m in range(mo2):
                nc.sync.dma_start(XO.ap()[:, m, :], xo_sb[:, m, :])
        else:
            nc.sync.dma_start(XO.ap(), xo_sb[:rpc, 0, :])
        ctx.close()
    return _finish(nc)


# ------------------------------------------------------------------ K4b
def build_k4b():
    """xU1 = relu(N2 @ z2 + b0) replicated;
    xU2[sl] = relu((N1[sl]@xd1 + N1[sl][:,perm1]@xU1) @ Wu1 + b1)."""
    nc = _new_nc()
    NT2 = nc.dram_tensor("NT2", [128, 8, 1024], BF16, kind="ExternalInput")
    Z2 = nc.dram_tensor("Z2", [128, 8, C], BF16, kind="ExternalInput")
    NT1B = nc.dram_tensor("NT1B", [128, 16, C], BF16, kind="ExternalInput")
    XD1 = nc.dram_tensor("XD1", [128, 16, C], BF16, kind="ExternalInput")
    Q1B = nc.dram_tensor("Q1B", [128, 8, C], BF16, kind="ExternalInput")
    WU1 = nc.dram_tensor("WU1", [128, 2, C], BF16, kind="ExternalInput")
    IDT = nc.dram_tensor("IDT", [128, 128], BF16, kind="ExternalInput")
    ONES = nc.dram_tensor("ONES", [1, 128], BF16, kind="ExternalInput")
    B0 = nc.dram_tensor("B0", [1, C], BF16, kind="ExternalInput")
    B1 = nc.dram_tensor("B1", [1, C], BF16, kind="ExternalInput")
    XO = nc.dram_tensor("XO", [128, 2, C], F32, kind="ExternalOutput")
    with tile.TileContext(nc) as tc:
        ctx = ExitStack()
        sb = ctx.enter_context(tc.tile_pool(name="sb", bufs=1))
        ps = ctx.enter_context(tc.tile_pool(name="ps", bufs=1, space="PSUM"))
        z2_sb = sb.tile([128, 8, C], BF16, tag="z2")
        nc.sync.dma_start(z2_sb[:], Z2.ap())
        ones_sb = sb.tile([128, 128], BF16, tag="ones")
        nc.sync.dma_start(ones_sb[:1, :], ONES.ap())
        b0_sb = sb.tile([128, C], BF16, tag="b0")
        nc.sync.dma_start(b0_sb[:1, :], B0.ap())
        b1_sb = sb.tile([128, C], BF16, tag="b1")
        nc.sync.dma_start(b1_sb[:1, :], B1.ap())
        idt_sb = sb.tile([128, 128], BF16, tag="idt")
        nc.sync.dma_start(idt_sb[:], IDT.ap())
        wu1_sb = sb.tile([128, 2, C], BF16, tag="wu1")
        nc.sync.dma_start(wu1_sb[:], WU1.ap())
        nt2_sb = sb.tile([128, 8, 1024], BF16, tag="nt2")
        for k0 in range(0, 8, 2):
            _ld(nc, nt2_sb, NT2, k0, k0 + 2)
        xd1_sb = sb.tile([128, 16, C], BF16, tag="xd1")
        for k0 in range(0, 16, 8):
            _ld(nc, xd1_sb, XD1, k0, k0 + 8)
        nt1b_sb = sb.tile([128, 16, C], BF16, tag="nt1b")
        for k0 in range(0, 16, 8):
            _ld(nc, nt1b_sb, NT1B, k0, k0 + 8)
        q1b_sb = sb.tile([128, 8, C], BF16, tag="q1b")
        nc.sync.dma_start(q1b_sb[:], Q1B.ap())

        xu1_sb = sb.tile([128, 8, C], BF16, tag="xu1")
        # v2 psums get dedicated banks so the xd1 part can accumulate while
        # xU1 is still being produced
        pv = [ps.tile([128, C], F32, tag=f"v{m}", name=f"v{m}")
              for m in range(2)]
        xu1_groups = [list(range(6)), [6, 7]]
        for mos in xu1_groups:
            pss = {m: ps.tile([128, C], F32, tag=f"p{m % 6}",
                              name=f"pu{m}") for m in mos}
            for kt in range(8):
                for mo in mos:
                    nc.tensor.matmul(
                        pss[mo][:], nt2_sb[:, kt, mo * 128:(mo + 1) * 128],
                        z2_sb[:, kt, :], start=(kt == 0), stop=False)
            for mo in mos:
                nc.tensor.matmul(pss[mo][:], ones_sb[:1, :], b0_sb[:1, :],
                                 start=False, stop=True)
                nc.vector.tensor_scalar_max(xu1_sb[:, mo, :], pss[mo][:],
                                            0.0)
            if mos[0] == 0:
                # xd1 part of v2 can start as soon as its inputs land
                for mo in range(2):
                    for kt in range(16):
                        nc.tensor.matmul(
                            pv[mo][:],
                            nt1b_sb[:, kt, mo * 128:(mo + 1) * 128],
                            xd1_sb[:, kt, :], start=(kt == 0), stop=False)
        v2_sb = sb.tile([128, 2, C], BF16, tag="v2")
        for mo in range(2):
            for kt in range(8):
                nc.tensor.matmul(
                    pv[mo][:], q1b_sb[:, kt, mo * 128:(mo + 1) * 128],
                    xu1_sb[:, kt, :], start=False, stop=(kt == 7))
            nc.any.tensor_copy(v2_sb[:, mo, :], pv[mo][:])
        v2t_sb = sb.tile([128, 2, C], BF16, tag="v2t")
        for mo in range(2):
            for cc in range(2):
                pst = ps.tile([128, 128], BF16, tag=f"p{2 + mo * 2 + cc}",
                              name="pt")
                nc.tensor.transpose(pst[:],
                                    v2_sb[:, mo, cc * 128:(cc + 1) * 128],
                                    idt_sb[:])
                nc.any.tensor_copy(v2t_sb[:, cc, mo * 128:(mo + 1) * 128],
                                   pst[:])
        xo_sb = sb.tile([128, 2, C], F32, tag="xo")
        for mo in range(2):
            ps3 = ps.tile([128, C], F32, tag=f"v{mo}", name="pw")
            for kt in range(2):
                nc.tensor.matmul(
                    ps3[:], v2t_sb[:, kt, mo * 128:(mo + 1) * 128],
                    wu1_sb[:, kt, :], start=(kt == 0), stop=False)
            nc.tensor.matmul(ps3[:], ones_sb[:1, :], b1_sb[:1, :],
                             start=False, stop=True)
            nc.vector.tensor_scalar_max(xo_sb[:, mo, :], ps3[:], 0.0)
        nc.sync.dma_start(XO.ap(), xo_sb[:])
        ctx.close()
    return _finish(nc)


# =================================================================== host
F8NP = ml_dtypes.float8_e4m3fn
BFNP = ml_dtypes.bfloat16


def _pm(a, dt):
    """[K, F] row-major -> partition-major [128, K//128, F]."""
    K, F = a.shape
    return np.ascontiguousarray(
        a.reshape(K // 128, 128, F).transpose(1, 0, 2)).astype(dt)


def _unpm(b):
    """[128, KT, F] -> [K, F]."""
    p, kt, f = b.shape
    return np.asarray(b, np.float32).transpose(1, 0, 2).reshape(kt * p, f)


def _pmv(v):
    """[K] -> [128, K//128] partition-major (padded to 128 rows)."""
    k = v.shape[0]
    if k < 128:
        v = np.pad(v, (0, 128 - k))
        k = 128
    return np.ascontiguousarray(
        v.reshape(k // 128, 128).T).astype(np.float32)


def _dgblk(c, dt):
    """[rpc] diag values -> [128, mo2, 128] block-diagonal lhsT."""
    rpc = c.shape[0]
    mo2 = (rpc + 127) // 128
    out = np.zeros((128, mo2, 128), np.float32)
    for m in range(mo2):
        seg = c[m * 128:(m + 1) * 128]
        out[np.arange(len(seg)), m, np.arange(len(seg))] = seg
    return out.astype(dt)


def _rowvec(v, width, dt):
    out = np.zeros((1, width), np.float32)
    out[0, :v.shape[0]] = v
    return out.astype(dt)


def _mk_dis(deg):
    return (1.0 / np.sqrt(np.maximum(deg, 1e-12))).astype(np.float32)


def _diag_inputs(A8T, z, dis, y, bvec, rpc):
    """Per-core in_maps for the K1/K4c program."""
    zhi = z.astype(F8NP)
    zlo = (z - zhi.astype(np.float32)).astype(F8NP)
    zhi_pm, zlo_pm = _pm(zhi, F8NP), _pm(zlo, F8NP)
    bv = np.asarray(bvec, np.float32)[None, :].astype(BFNP)
    maps = []
    for c in range(NCORES):
        sl = slice(c * rpc, (c + 1) * rpc)
        maps.append({
            "AT": np.ascontiguousarray(A8T[:, :, sl]),
            "ZH": zhi_pm, "ZL": zlo_pm,
            "DISP": _pmv(dis[sl]),
            "DG": _dgblk(2.0 * dis[sl], BFNP),
            "YS": _pm(y[sl].astype(np.float32), BFNP),
            "IV": _rowvec(1.0 / dis[sl], rpc, BFNP),
            "BV": bv,
            })
    return maps


def kernel(x, edge_index, W_init, b_init, W_down, b_down, p_pool,
           W_up, b_up, W_final, b_final):
    x = np.asarray(x, np.float32)
    N = x.shape[0]
    rpc0 = N // NCORES

    A0 = np.zeros((N, N), np.float32)
    np.add.at(A0, (np.asarray(edge_index[0]), np.asarray(edge_index[1])),
              1.0)
    dis0 = _mk_dis(A0.sum(1) + 2.0)
    y0 = x @ np.asarray(W_init, np.float32)

    # exact level-0 score via host matvec (init GCN is linear)
    p0 = np.asarray(p_pool[0], np.float32)
    u = y0 @ p0
    s0 = (dis0 * (A0 @ (dis0 * u)) + 2.0 * dis0 * dis0 * u) \
        / np.linalg.norm(p0)
    perm0 = np.argsort(-s0, kind="stable")[:N // 2]
    sv0 = s0[perm0]

    # ---- K1
    A8Tpm = _pm(np.ascontiguousarray(A0.T), F8NP)  # [128, 32, 4096]
    nc1 = build_diag()
    maps = _diag_inputs(A8Tpm, dis0[:, None] * y0, dis0, y0,
                        np.asarray(b_init, np.float32), rpc0)
    res = _run(nc1, maps)
    x0 = np.concatenate([_unpm(r["XO"]) for r in res], 0)

    # ---- down levels
    Bh = A0 + np.eye(N, dtype=np.float32)
    xcur, perm, sv = x0, perm0, sv0
    n = N
    Ms, dis_l, xs, perms = [], [dis0], [x0], []
    level_fp8 = [True, True, False]
    for lev in range(3):
        k = n // 2
        rpc = k // NCORES
        perms.append(perm)
        L = Bh[perm, :]
        R = Bh[:, perm]
        lim = 16 if level_fp8[lev] else 256
        assert Bh.max() <= lim, (lev, Bh.max())
        diagM = np.einsum('it,ti->i', L, R, optimize=True)
        deg = L @ R.sum(1) - diagM + 2.0
        dis = _mk_dis(deg)
        xp = xcur[perm] * np.tanh(sv)[:, None]
        y = xp @ np.asarray(W_down[lev], np.float32)
        adt = FP8 if level_fp8[lev] else BF16
        npdt = NP_OF[adt]
        ship = lev < 2
        ydt = BFNP if ship else np.float32
        nc = build_level(n, k, rpc, adt, ship)
        Rpm = _pm(R, npdt)
        yfull = (dis[:, None] * y).astype(np.float32)
        yp_pm = _pm(yfull, ydt)
        bvec = np.asarray(b_down[lev], np.float32)
        maps = []
        for cc in range(NCORES):
            sl = slice(cc * rpc, (cc + 1) * rpc)
            maps.append({
                "R": Rpm,
                "LT": _pm(np.ascontiguousarray(L[sl].T), npdt),
                "YP": yp_pm,
                "DISP": _pmv(dis[sl]),
                "DG": _dgblk(2.0 - diagM[sl], ydt),
                "YS": _pm(yfull[sl], ydt) if rpc >= 128 else
                np.ascontiguousarray(
                    np.pad(yfull[sl], ((0, 128 - rpc), (0, 0)))
                    [:, None, :]).astype(ydt),
                "IV": _rowvec(1.0 / dis[sl], max(rpc, 128), ydt),
                "BV": bvec[None, :].astype(ydt),
                })
        res = _run(nc, maps)
        if rpc >= 128:
            xn = np.concatenate([_unpm(r["XO"]) for r in res], 0)
        else:
            xn = np.concatenate([np.asarray(r["XO"], np.float32)
                                 for r in res], 0)
        if ship:
            M = np.concatenate([_unpm(r["MT"]).T for r in res], 0)
            Ms.append(M)
            Bh = M.copy()
            np.fill_diagonal(Bh, 1.0)
        dis_l.append(dis)
        xs.append(xn)
        xcur, n = xn, k
        if lev < 2:
            pl = np.asarray(p_pool[lev + 1], np.float32)
            s = xn @ pl / np.linalg.norm(pl)
            perm = np.argsort(-s, kind="stable")[:k // 2]
            sv = s[perm]

    x_d1, x_d2, x_d3 = xs[1], xs[2], xs[3]
    dis1, dis2 = dis_l[1], dis_l[2]
    M1, M2 = Ms[0], Ms[1]
    perm1, perm2 = perms[1], perms[2]

    # ---- K4b
    N2 = M2.copy()
    np.fill_diagonal(N2, 2.0)
    N2 *= dis2[:, None] * dis2[None, :]
    N1 = M1.copy()
    np.fill_diagonal(N1, 2.0)
    N1 *= dis1[:, None] * dis1[None, :]
    up = np.zeros_like(x_d2)
    up[perm2] = x_d3
    z2 = (x_d2 + up) @ np.asarray(W_up[0], np.float32)
    nc4b = build_k4b()
    rpc1 = 2048 // NCORES
    nt2_pm = _pm(np.ascontiguousarray(N2.T), BFNP)
    z2_pm = _pm(z2, BFNP)
    xd1_pm = _pm(x_d1, BFNP)
    wu1_pm = _pm(np.asarray(W_up[1], np.float32), BFNP)
    idt = np.eye(128, dtype=np.float32).astype(BFNP)
    ones = np.ones((1, 128), BFNP)
    b0 = np.asarray(b_up[0], np.float32)[None, :].astype(BFNP)
    b1 = np.asarray(b_up[1], np.float32)[None, :].astype(BFNP)
    maps = []
    for cc in range(NCORES):
        sl = slice(cc * rpc1, (cc + 1) * rpc1)
        maps.append({
            "NT2": nt2_pm, "Z2": z2_pm,
            "NT1B": _pm(np.ascontiguousarray(N1[sl].T), BFNP),
            "XD1": xd1_pm,
            "Q1B": _pm(np.ascontiguousarray(N1[sl][:, perm1].T), BFNP),
            "WU1": wu1_pm, "IDT": idt, "ONES": ones, "B0": b0, "B1": b1,
            })
    res = _run(nc4b, maps)
    xU2 = np.concatenate([_unpm(r["XO"]) for r in res], 0)

    # ---- K4c
    upf = np.zeros_like(x0)
    upf[perm0] = xU2
    zf = (x0 + upf) @ np.asarray(W_final, np.float32)
    nc4c = build_diag()
    maps = _diag_inputs(A8Tpm, dis0[:, None] * zf, dis0, zf,
                        np.asarray(b_final, np.float32), rpc0)
    res = _run(nc4c, maps)
    out = np.concatenate([_unpm(r["XO"]) for r in res], 0)
    return out.astype(np.float32)


# revision 17
# speedup vs baseline: 1.5382x; 1.1145x over previous
"""GraphUNet (N=4096, E=65536, C=256, depth 3, ratio 0.5) on 8 trn2 NeuronCores.

Row-sharded SPMD pipeline, 6 launches. Device does the O(n^2 C) message
passing (N @ Z SpMMs) and the dense A@A augmentations; host does top-k,
gathers, O(nC^2) feature transforms / weight folds, and O(n^2)
element-wise adjacency prep between launches.

Layout/overlap notes: every DRAM tensor ships partition-major
([128, K/128, F]) so DMA descriptors move >=512B contiguous runs at the
full modeled 360 GB/s; input DMAs are emitted in consumption order with
the streamed operand chunk-interleaved so the PE starts ~4us in; output
DMAs are emitted last (the DMA queue is in-order, so a store's semaphore
wait would block later loads); diag/bias GCN corrections are folded into
the PSUM accumulation as rank-1 / diagonal matmul chains, leaving one
fused tensor_scalar per output block.

  K1   init GCN:   x0[sl] = dis0*(A0[sl] @ (zhi+zlo) + 2*dis0*y0[sl]
                   + (1/dis0) x b), A0 in exact small-int fp8 (DoubleRow),
                   z = dis0*(x@Wi) as an fp8 hi+lo split.
  K2-K4a levels:   M^T col-block = R^T @ L[sl]^T chain in fp8 (exact ints;
                   bf16 at level 3), shipped back raw (bf16-exact ints);
                   the down GCN reuses the in-SBUF M^T block as lhsT
                   against host-prescaled yp = dis*(xp@Wd), with the
                   diag/bias corrections in-chain and relu fused into the
                   dis row-scale consumer.
  K4b  up GCNs:    xU1 = relu(N2 @ z2 + b) replicated (z2 host-folds the
                   deepest unpool scatter), then xU2[sl] = relu((N1[sl]@xd1
                   + N1[sl][:,perm1]@xU1) @ Wu1 + b) with one on-device
                   transpose; biases enter the psum via rank-1 ones chains.
  K4c  final GCN:  identical program to K1 on zf = (x0 + scatter(perm0,
                   xU2)) @ Wf -- the unpool scatter and Wf fold on host
                   collapse the two chains of the reference into one.

Precision: adjacency chains exact; features bf16 (fp8 hi+lo where they
feed the big A-chains); measured end-to-end rel err ~7e-3 vs f32 ref.
"""

import numpy as np
import ml_dtypes

from contextlib import ExitStack

import concourse.bass as bass
import concourse.mybir as mybir
import concourse.tile as tile
from concourse import bacc
from concourse.bass_utils import run_bass_kernel_spmd

NCORES = 8
C = 256
F32 = mybir.dt.float32
F32R = mybir.dt.float32r
BF16 = mybir.dt.bfloat16
FP8 = mybir.dt.float8e4

NP_OF = {F32: np.float32, F32R: np.float32,
         BF16: ml_dtypes.bfloat16, FP8: ml_dtypes.float8_e4m3fn}

_TRACE = {"on": False, "results": [], "ncs": []}
DR = mybir.MatmulPerfMode.DoubleRow
MULT = mybir.AluOpType.mult
MAXOP = mybir.AluOpType.max


def _new_nc():
    return bacc.Bacc("TRN2", target_bir_lowering=False, debug=False,
                     num_devices=NCORES)


def _finish(nc):
    nc.compile()
    _TRACE["ncs"].append(nc)
    return nc


def _run(nc, in_maps):
    res = run_bass_kernel_spmd(nc, in_maps, list(range(NCORES)),
                               trace=_TRACE["on"])
    if _TRACE["on"]:
        _TRACE["results"].append(res)
    return res.results


def _ld(nc, t, dram, k0, k1, f0=None, f1=None):
    if f0 is None:
        nc.sync.dma_start(t[:, k0:k1, :], dram.ap()[:, k0:k1, :])
    else:
        nc.sync.dma_start(t[:, k0:k1, f0:f1], dram.ap()[:, k0:k1, f0:f1])


# ------------------------------------------------------------ K1 / K4c
def build_diag():
    """x[sl] = dis[sl]*(A0[sl] @ (zhi+zlo) + diag(2 dis[sl]) y[sl]
    + (1/dis[sl]) x b).   [4096 -> 512/core]

    AT ships mo-major so each 128-row output chain completes as soon as
    its A-slice lands; consumers and stores pipeline behind the PE."""
    n, rpc, KT, MO = 4096, 512, 32, 4
    nc = _new_nc()
    AT = nc.dram_tensor("AT", [128, MO, KT, 128], FP8,
                        kind="ExternalInput")
    ZH = nc.dram_tensor("ZH", [128, KT, C], FP8, kind="ExternalInput")
    ZL = nc.dram_tensor("ZL", [128, KT, C], FP8, kind="ExternalInput")
    DISP = nc.dram_tensor("DISP", [128, MO], F32, kind="ExternalInput")
    DG = nc.dram_tensor("DG", [128, MO, 128], BF16, kind="ExternalInput")
    YS = nc.dram_tensor("YS", [128, MO, C], BF16, kind="ExternalInput")
    IV = nc.dram_tensor("IV", [1, rpc], BF16, kind="ExternalInput")
    BV = nc.dram_tensor("BV", [1, C], BF16, kind="ExternalInput")
    XO = nc.dram_tensor("XO", [128, MO, C], F32, kind="ExternalOutput")
    with tile.TileContext(nc) as tc:
        ctx = ExitStack()
        sb = ctx.enter_context(tc.tile_pool(name="sb", bufs=1))
        ps = ctx.enter_context(tc.tile_pool(name="ps", bufs=1, space="PSUM"))
        at_sb = sb.tile([128, MO, KT, 128], FP8, tag="at")
        zh_sb = sb.tile([128, KT, C], FP8, tag="zh")
        zl_sb = sb.tile([128, KT, C], FP8, tag="zl")
        disp_sb = sb.tile([128, MO], F32, tag="disp")
        dg_sb = sb.tile([128, MO, 128], BF16, tag="dg")
        ys_sb = sb.tile([128, MO, C], BF16, tag="ys")
        iv_sb = sb.tile([128, rpc], BF16, tag="iv")
        bv_sb = sb.tile([128, C], BF16, tag="bv")
        nc.sync.dma_start(at_sb[:, 0, :, :], AT.ap()[:, 0, :, :])
        for k0 in range(0, KT, 16):
            _ld(nc, zh_sb, ZH, k0, k0 + 16)
            _ld(nc, zl_sb, ZL, k0, k0 + 16)
        nc.sync.dma_start(at_sb[:, 1, :, :], AT.ap()[:, 1, :, :])
        nc.sync.dma_start(disp_sb[:], DISP.ap())
        nc.sync.dma_start(dg_sb[:], DG.ap())
        nc.sync.dma_start(ys_sb[:], YS.ap())
        nc.sync.dma_start(iv_sb[:1, :], IV.ap())
        nc.sync.dma_start(bv_sb[:1, :], BV.ap())
        nc.sync.dma_start(at_sb[:, 2, :, :], AT.ap()[:, 2, :, :])
        nc.sync.dma_start(at_sb[:, 3, :, :], AT.ap()[:, 3, :, :])
        xo_sb = sb.tile([128, MO, C], F32, tag="xo")
        for mo in range(MO):
            pso = ps.tile([128, C], F32, tag=f"p{mo}", name=f"p{mo}")
            for kp in range(KT // 2):
                for ci, ch in enumerate((zh_sb, zl_sb)):
                    nc.tensor.matmul(
                        pso[:], at_sb[:, mo, 2 * kp:2 * kp + 2, :],
                        ch[:, 2 * kp:2 * kp + 2, :],
                        start=(kp == 0 and ci == 0), stop=False,
                        perf_mode=DR)
            nc.tensor.matmul(pso[:], dg_sb[:, mo, :], ys_sb[:, mo, :],
                             start=False, stop=False)
            nc.tensor.matmul(pso[:],
                             iv_sb[:1, mo * 128:(mo + 1) * 128],
                             bv_sb[:1, :], start=False, stop=True)
            nc.any.tensor_scalar_mul(xo_sb[:, mo, :], pso[:],
                                     disp_sb[:, mo:mo + 1])
        for mo in range(MO):
            nc.sync.dma_start(XO.ap()[:, mo, :], xo_sb[:, mo, :])
        ctx.close()
    return _finish(nc)


# ----------------------------------------------------- K2 / K3 / K4a
def build_level(npv, n, rpc, adt, ship):
    """M^T col-block = R^T @ L[sl]^T (exact ints), then
    x[sl] = relu(dis[sl] * (M[sl] @ yp + diag(2-diagM) yp[sl]
    + (1/dis) x b)).  Ships raw M^T if `ship`."""
    KTp, KT = npv // 128, n // 128
    mo2 = (rpc + 127) // 128
    mdt = BF16 if ship else F32R
    ydt = BF16 if ship else F32R
    nc = _new_nc()
    R = nc.dram_tensor("R", [128, KTp, n], adt, kind="ExternalInput")
    LT = nc.dram_tensor("LT", [128, KTp, rpc], adt, kind="ExternalInput")
    YP = nc.dram_tensor("YP", [128, KT, C], ydt, kind="ExternalInput")
    DISP = nc.dram_tensor("DISP", [128, mo2], F32, kind="ExternalInput")
    DG = nc.dram_tensor("DG", [128, mo2, 128], ydt, kind="ExternalInput")
    YS = nc.dram_tensor("YS", [128, mo2, C], ydt, kind="ExternalInput")
    IV = nc.dram_tensor("IV", [1, max(rpc, 128)], ydt,
                        kind="ExternalInput")
    BV = nc.dram_tensor("BV", [1, C], ydt, kind="ExternalInput")
    if ship:
        MT = nc.dram_tensor("MT", [128, KT, rpc], BF16,
                            kind="ExternalOutput")
    XO = nc.dram_tensor("XO", [128, mo2, C] if rpc >= 128 else [rpc, C],
                        F32, kind="ExternalOutput")
    groups = [list(range(g, min(g + 6, KT))) for g in range(0, KT, 6)]
    with tile.TileContext(nc) as tc:
        ctx = ExitStack()
        sb = ctx.enter_context(tc.tile_pool(name="sb", bufs=1))
        ps = ctx.enter_context(tc.tile_pool(name="ps", bufs=1, space="PSUM"))
        disp_sb = sb.tile([128, mo2], F32, tag="disp")
        dg_sb = sb.tile([128, mo2, 128], ydt, tag="dg")
        ys_sb = sb.tile([128, mo2, C], ydt, tag="ys")
        iv_sb = sb.tile([128, max(rpc, 128)], ydt, tag="iv")
        bv_sb = sb.tile([128, C], ydt, tag="bv")
        lt_sb = sb.tile([128, KTp, rpc], adt, tag="lt")
        for k0 in range(0, KTp, 8):
            _ld(nc, lt_sb, LT, k0, min(KTp, k0 + 8))
        r_sb = sb.tile([128, KTp, n], adt, tag="r")
        yp_sb = sb.tile([128, KT, C], ydt, tag="yp")
        for gi, mos in enumerate(groups):
            f0, f1 = mos[0] * 128, (mos[-1] + 1) * 128
            for k0 in range(0, KTp, 8):
                _ld(nc, r_sb, R, k0, min(KTp, k0 + 8), f0, f1)
            if gi == 0:
                nc.sync.dma_start(disp_sb[:], DISP.ap())
                nc.sync.dma_start(dg_sb[:], DG.ap())
                nc.sync.dma_start(ys_sb[:], YS.ap())
                nc.sync.dma_start(iv_sb[:1, :], IV.ap())
                nc.sync.dma_start(bv_sb[:1, :], BV.ap())
                for k0 in range(0, KT, 8):
                    _ld(nc, yp_sb, YP, k0, min(KT, k0 + 8))
        mt_sb = sb.tile([128, KT, rpc], mdt, tag="mt")
        xo_sb = sb.tile([128, mo2, C], F32, tag="xo")
        use_dr = adt == FP8
        gps = [ps.tile([128, C], F32, tag=f"g{m}", name=f"g{m}")
               for m in range(mo2)]
        msz2 = min(128, rpc)
        for gi, mos in enumerate(groups):
            pss = {m: ps.tile([128, rpc], F32, tag=f"p{m % 6}",
                              name=f"p{m}") for m in mos}
            if use_dr:
                for kp in range(KTp // 2):
                    for mo in mos:
                        nc.tensor.matmul(
                            pss[mo][:],
                            r_sb[:, 2 * kp:2 * kp + 2,
                                 mo * 128:(mo + 1) * 128],
                            lt_sb[:, 2 * kp:2 * kp + 2, :],
                            start=(kp == 0), stop=(kp == KTp // 2 - 1),
                            perf_mode=DR)
            else:
                for kt in range(KTp):
                    for mo in mos:
                        nc.tensor.matmul(
                            pss[mo][:],
                            r_sb[:, kt, mo * 128:(mo + 1) * 128],
                            lt_sb[:, kt, :],
                            start=(kt == 0), stop=(kt == KTp - 1))
            for mo in mos:
                nc.any.tensor_copy(mt_sb[:, mo, :], pss[mo][:])
            for m in range(mo2):
                for kt in mos:
                    nc.tensor.matmul(
                        gps[m][:msz2, :],
                        mt_sb[:, kt, m * 128:m * 128 + msz2],
                        yp_sb[:, kt, :], start=(kt == 0), stop=False)
        for m in range(mo2):
            nc.tensor.matmul(gps[m][:msz2, :], dg_sb[:msz2, m, :msz2],
                             ys_sb[:msz2, m, :], start=False, stop=False)
            nc.tensor.matmul(gps[m][:msz2, :],
                             iv_sb[:1, m * 128:m * 128 + msz2],
                             bv_sb[:1, :], start=False, stop=True)
            nc.vector.tensor_scalar(xo_sb[:msz2, m, :], gps[m][:msz2, :],
                                    disp_sb[:msz2, m:m + 1], 0.0,
                                    MULT, MAXOP)
        if ship:
            for gi, mos in enumerate(groups):
                nc.sync.dma_start(MT.ap()[:, mos[0]:mos[-1] + 1, :],
                                  mt_sb[:, mos[0]:mos[-1] + 1, :])
        if rpc >= 128:
            for m in range(mo2):
                nc.sync.dma_start(XO.ap()[:, m, :], xo_sb[:, m, :])
        else:
            nc.sync.dma_start(XO.ap(), xo_sb[:rpc, 0, :])
        ctx.close()
    return _finish(nc)


# ------------------------------------------------------------------ K4b
def build_k4b():
    """xU1 = relu(N2 @ z2 + b0) replicated;
    xU2[sl] = relu((N1[sl]@xd1 + N1[sl][:,perm1]@xU1) @ Wu1 + b1)."""
    nc = _new_nc()
    NT2 = nc.dram_tensor("NT2", [128, 8, 1024], BF16, kind="ExternalInput")
    Z2 = nc.dram_tensor("Z2", [128, 8, C], BF16, kind="ExternalInput")
    NT1B = nc.dram_tensor("NT1B", [128, 16, C], BF16, kind="ExternalInput")
    XD1 = nc.dram_tensor("XD1", [128, 16, C], BF16, kind="ExternalInput")
    Q1B = nc.dram_tensor("Q1B", [128, 8, C], BF16, kind="ExternalInput")
    WU1 = nc.dram_tensor("WU1", [128, 2, C], BF16, kind="ExternalInput")
    IDT = nc.dram_tensor("IDT", [128, 128], BF16, kind="ExternalInput")
    ONES = nc.dram_tensor("ONES", [1, 128], BF16, kind="ExternalInput")
    B0 = nc.dram_tensor("B0", [1, C], BF16, kind="ExternalInput")
    B1 = nc.dram_tensor("B1", [1, C], BF16, kind="ExternalInput")
    XO = nc.dram_tensor("XO", [128, 2, C], F32, kind="ExternalOutput")
    with tile.TileContext(nc) as tc:
        ctx = ExitStack()
        sb = ctx.enter_context(tc.tile_pool(name="sb", bufs=1))
        ps = ctx.enter_context(tc.tile_pool(name="ps", bufs=1, space="PSUM"))
        z2_sb = sb.tile([128, 8, C], BF16, tag="z2")
        nc.sync.dma_start(z2_sb[:], Z2.ap())
        nt2_sb = sb.tile([128, 8, 1024], BF16, tag="nt2")
        for k0 in range(0, 8, 2):
            _ld(nc, nt2_sb, NT2, k0, k0 + 2)
        ones_sb = sb.tile([128, 128], BF16, tag="ones")
        nc.sync.dma_start(ones_sb[:1, :], ONES.ap())
        b0_sb = sb.tile([128, C], BF16, tag="b0")
        nc.sync.dma_start(b0_sb[:1, :], B0.ap())
        b1_sb = sb.tile([128, C], BF16, tag="b1")
        nc.sync.dma_start(b1_sb[:1, :], B1.ap())
        idt_sb = sb.tile([128, 128], BF16, tag="idt")
        nc.sync.dma_start(idt_sb[:], IDT.ap())
        wu1_sb = sb.tile([128, 2, C], BF16, tag="wu1")
        nc.sync.dma_start(wu1_sb[:], WU1.ap())
        xd1_sb = sb.tile([128, 16, C], BF16, tag="xd1")
        for k0 in range(0, 16, 8):
            _ld(nc, xd1_sb, XD1, k0, k0 + 8)
        nt1b_sb = sb.tile([128, 16, C], BF16, tag="nt1b")
        for k0 in range(0, 16, 8):
            _ld(nc, nt1b_sb, NT1B, k0, k0 + 8)
        q1b_sb = sb.tile([128, 8, C], BF16, tag="q1b")
        nc.sync.dma_start(q1b_sb[:], Q1B.ap())

        xu1_sb = sb.tile([128, 8, C], BF16, tag="xu1")
        # v2 psums get dedicated banks so the xd1 part can accumulate while
        # xU1 is still being produced
        pv = [ps.tile([128, C], F32, tag=f"v{m}", name=f"v{m}")
              for m in range(2)]
        xu1_groups = [list(range(6)), [6, 7]]
        for mos in xu1_groups:
            pss = {m: ps.tile([128, C], F32, tag=f"p{m % 6}",
                              name=f"pu{m}") for m in mos}
            for kt in range(8):
                for mo in mos:
                    nc.tensor.matmul(
                        pss[mo][:], nt2_sb[:, kt, mo * 128:(mo + 1) * 128],
                        z2_sb[:, kt, :], start=(kt == 0), stop=False)
            for mo in mos:
                nc.tensor.matmul(pss[mo][:], ones_sb[:1, :], b0_sb[:1, :],
                                 start=False, stop=True)
                nc.vector.tensor_scalar_max(xu1_sb[:, mo, :], pss[mo][:],
                                            0.0)
            if mos[0] == 0:
                # xd1 part of v2 can start as soon as its inputs land
                for mo in range(2):
                    for kt in range(16):
                        nc.tensor.matmul(
                            pv[mo][:],
                            nt1b_sb[:, kt, mo * 128:(mo + 1) * 128],
                            xd1_sb[:, kt, :], start=(kt == 0), stop=False)
        v2_sb = sb.tile([128, 2, C], BF16, tag="v2")
        for mo in range(2):
            for kt in range(8):
                nc.tensor.matmul(
                    pv[mo][:], q1b_sb[:, kt, mo * 128:(mo + 1) * 128],
                    xu1_sb[:, kt, :], start=False, stop=(kt == 7))
            nc.any.tensor_copy(v2_sb[:, mo, :], pv[mo][:])
        v2t_sb = sb.tile([128, 2, C], BF16, tag="v2t")
        for mo in range(2):
            for cc in range(2):
                pst = ps.tile([128, 128], BF16, tag=f"p{2 + mo * 2 + cc}",
                              name="pt")
                nc.tensor.transpose(pst[:],
                                    v2_sb[:, mo, cc * 128:(cc + 1) * 128],
                                    idt_sb[:])
                nc.any.tensor_copy(v2t_sb[:, cc, mo * 128:(mo + 1) * 128],
                                   pst[:])
        xo_sb = sb.tile([128, 2, C], F32, tag="xo")
        for mo in range(2):
            ps3 = ps.tile([128, C], F32, tag=f"v{mo}", name="pw")
            for kt in range(2):
                nc.tensor.matmul(
                    ps3[:], v2t_sb[:, kt, mo * 128:(mo + 1) * 128],
                    wu1_sb[:, kt, :], start=(kt == 0), stop=False)
            nc.tensor.matmul(ps3[:], ones_sb[:1, :], b1_sb[:1, :],
                             start=False, stop=True)
            nc.vector.tensor_scalar_max(xo_sb[:, mo, :], ps3[:], 0.0)
        nc.sync.dma_start(XO.ap(), xo_sb[:])
        ctx.close()
    return _finish(nc)


# =================================================================== host
F8NP = ml_dtypes.float8_e4m3fn
BFNP = ml_dtypes.bfloat16


def _pm(a, dt):
    """[K, F] row-major -> partition-major [128, K//128, F]."""
    K, F = a.shape
    return np.ascontiguousarray(
        a.reshape(K // 128, 128, F).transpose(1, 0, 2)).astype(dt)


def _unpm(b):
    """[128, KT, F] -> [K, F]."""
    p, kt, f = b.shape
    return np.asarray(b, np.float32).transpose(1, 0, 2).reshape(kt * p, f)


def _pmv(v):
    """[K] -> [128, K//128] partition-major (padded to 128 rows)."""
    k = v.shape[0]
    if k < 128:
        v = np.pad(v, (0, 128 - k))
        k = 128
    return np.ascontiguousarray(
        v.reshape(k // 128, 128).T).astype(np.float32)


def _dgblk(c, dt):
    """[rpc] diag values -> [128, mo2, 128] block-diagonal lhsT."""
    rpc = c.shape[0]
    mo2 = (rpc + 127) // 128
    out = np.zeros((128, mo2, 128), np.float32)
    for m in range(mo2):
        seg = c[m * 128:(m + 1) * 128]
        out[np.arange(len(seg)), m, np.arange(len(seg))] = seg
    return out.astype(dt)


def _rowvec(v, width, dt):
    out = np.zeros((1, width), np.float32)
    out[0, :v.shape[0]] = v
    return out.astype(dt)


def _mk_dis(deg):
    return (1.0 / np.sqrt(np.maximum(deg, 1e-12))).astype(np.float32)


def _diag_inputs(A8T, z, dis, y, bvec, rpc):
    """Per-core in_maps for the K1/K4c program. A8T is [4096, 4096] fp8
    (= A0^T); the per-core AT block ships mo-major [128, MO, KT, 128]."""
    zhi = z.astype(F8NP)
    zlo = (z - zhi.astype(np.float32)).astype(F8NP)
    zhi_pm, zlo_pm = _pm(zhi, F8NP), _pm(zlo, F8NP)
    bv = np.asarray(bvec, np.float32)[None, :].astype(BFNP)
    n = A8T.shape[0]
    maps = []
    for c in range(NCORES):
        sl = slice(c * rpc, (c + 1) * rpc)
        blk = A8T[:, sl]                       # [n, rpc]
        at = np.ascontiguousarray(
            blk.reshape(n // 128, 128, rpc // 128, 128)
            .transpose(1, 2, 0, 3))            # [128, MO, KT, 128]
        maps.append({
            "AT": at,
            "ZH": zhi_pm, "ZL": zlo_pm,
            "DISP": _pmv(dis[sl]),
            "DG": _dgblk(2.0 * dis[sl], BFNP),
            "YS": _pm(y[sl].astype(np.float32), BFNP),
            "IV": _rowvec(1.0 / dis[sl], rpc, BFNP),
            "BV": bv,
            })
    return maps


def kernel(x, edge_index, W_init, b_init, W_down, b_down, p_pool,
           W_up, b_up, W_final, b_final):
    x = np.asarray(x, np.float32)
    N = x.shape[0]
    rpc0 = N // NCORES

    A0 = np.zeros((N, N), np.float32)
    np.add.at(A0, (np.asarray(edge_index[0]), np.asarray(edge_index[1])),
              1.0)
    dis0 = _mk_dis(A0.sum(1) + 2.0)
    y0 = x @ np.asarray(W_init, np.float32)

    # exact level-0 score via host matvec (init GCN is linear)
    p0 = np.asarray(p_pool[0], np.float32)
    u = y0 @ p0
    s0 = (dis0 * (A0 @ (dis0 * u)) + 2.0 * dis0 * dis0 * u) \
        / np.linalg.norm(p0)
    perm0 = np.argsort(-s0, kind="stable")[:N // 2]
    sv0 = s0[perm0]

    # ---- K1
    A8T = np.ascontiguousarray(A0.T).astype(F8NP)  # [4096, 4096]
    nc1 = build_diag()
    maps = _diag_inputs(A8T, dis0[:, None] * y0, dis0, y0,
                        np.asarray(b_init, np.float32), rpc0)
    res = _run(nc1, maps)
    x0 = np.concatenate([_unpm(r["XO"]) for r in res], 0)

    # ---- down levels
    Bh = A0 + np.eye(N, dtype=np.float32)
    xcur, perm, sv = x0, perm0, sv0
    n = N
    Ms, dis_l, xs, perms = [], [dis0], [x0], []
    level_fp8 = [True, True, False]
    for lev in range(3):
        k = n // 2
        rpc = k // NCORES
        perms.append(perm)
        L = Bh[perm, :]
        R = Bh[:, perm]
        lim = 16 if level_fp8[lev] else 256
        assert Bh.max() <= lim, (lev, Bh.max())
        diagM = np.einsum('it,ti->i', L, R, optimize=True)
        deg = L @ R.sum(1) - diagM + 2.0
        dis = _mk_dis(deg)
        xp = xcur[perm] * np.tanh(sv)[:, None]
        y = xp @ np.asarray(W_down[lev], np.float32)
        adt = FP8 if level_fp8[lev] else BF16
        npdt = NP_OF[adt]
        ship = lev < 2
        ydt = BFNP if ship else np.float32
        nc = build_level(n, k, rpc, adt, ship)
        Rpm = _pm(R, npdt)
        yfull = (dis[:, None] * y).astype(np.float32)
        yp_pm = _pm(yfull, ydt)
        bvec = np.asarray(b_down[lev], np.float32)
        maps = []
        for cc in range(NCORES):
            sl = slice(cc * rpc, (cc + 1) * rpc)
            maps.append({
                "R": Rpm,
                "LT": _pm(np.ascontiguousarray(L[sl].T), npdt),
                "YP": yp_pm,
                "DISP": _pmv(dis[sl]),
                "DG": _dgblk(2.0 - diagM[sl], ydt),
                "YS": _pm(yfull[sl], ydt) if rpc >= 128 else
                np.ascontiguousarray(
                    np.pad(yfull[sl], ((0, 128 - rpc), (0, 0)))
                    [:, None, :]).astype(ydt),
                "IV": _rowvec(1.0 / dis[sl], max(rpc, 128), ydt),
                "BV": bvec[None, :].astype(ydt),
                })
        res = _run(nc, maps)
        if rpc >= 128:
            xn = np.concatenate([_unpm(r["XO"]) for r in res], 0)
        else:
            xn = np.concatenate([np.asarray(r["XO"], np.float32)
                                 for r in res], 0)
        if ship:
            M = np.concatenate([_unpm(r["MT"]).T for r in res], 0)
            Ms.append(M)
            Bh = M.copy()
            np.fill_diagonal(Bh, 1.0)
        dis_l.append(dis)
        xs.append(xn)
        xcur, n = xn, k
        if lev < 2:
            pl = np.asarray(p_pool[lev + 1], np.float32)
            s = xn @ pl / np.linalg.norm(pl)
            perm = np.argsort(-s, kind="stable")[:k // 2]
            sv = s[perm]

    x_d1, x_d2, x_d3 = xs[1], xs[2], xs[3]
    dis1, dis2 = dis_l[1], dis_l[2]
    M1, M2 = Ms[0], Ms[1]
    perm1, perm2 = perms[1], perms[2]

    # ---- K4b
    N2 = M2.copy()
    np.fill_diagonal(N2, 2.0)
    N2 *= dis2[:, None] * dis2[None, :]
    N1 = M1.copy()
    np.fill_diagonal(N1, 2.0)
    N1 *= dis1[:, None] * dis1[None, :]
    up = np.zeros_like(x_d2)
    up[perm2] = x_d3
    z2 = (x_d2 + up) @ np.asarray(W_up[0], np.float32)
    nc4b = build_k4b()
    rpc1 = 2048 // NCORES
    nt2_pm = _pm(np.ascontiguousarray(N2.T), BFNP)
    z2_pm = _pm(z2, BFNP)
    xd1_pm = _pm(x_d1, BFNP)
    wu1_pm = _pm(np.asarray(W_up[1], np.float32), BFNP)
    idt = np.eye(128, dtype=np.float32).astype(BFNP)
    ones = np.ones((1, 128), BFNP)
    b0 = np.asarray(b_up[0], np.float32)[None, :].astype(BFNP)
    b1 = np.asarray(b_up[1], np.float32)[None, :].astype(BFNP)
    maps = []
    for cc in range(NCORES):
        sl = slice(cc * rpc1, (cc + 1) * rpc1)
        maps.append({
            "NT2": nt2_pm, "Z2": z2_pm,
            "NT1B": _pm(np.ascontiguousarray(N1[sl].T), BFNP),
            "XD1": xd1_pm,
            "Q1B": _pm(np.ascontiguousarray(N1[sl][:, perm1].T), BFNP),
            "WU1": wu1_pm, "IDT": idt, "ONES": ones, "B0": b0, "B1": b1,
            })
    res = _run(nc4b, maps)
    xU2 = np.concatenate([_unpm(r["XO"]) for r in res], 0)

    # ---- K4c
    upf = np.zeros_like(x0)
    upf[perm0] = xU2
    zf = (x0 + upf) @ np.asarray(W_final, np.float32)
    nc4c = build_diag()
    maps = _diag_inputs(A8T, dis0[:, None] * zf, dis0, zf,
                        np.asarray(b_final, np.float32), rpc0)
    res = _run(nc4c, maps)
    out = np.concatenate([_unpm(r["XO"]) for r in res], 0)
    return out.astype(np.float32)
